# revision 5
# baseline (speedup 1.0000x reference)
"""Trainium2 Bass kernel for nn_MultiHeadPiiModel (segment_reduce).

Data-parallel over batch: 8 NeuronCores, one batch element each.
Per-core pipeline (all matmuls in fp32r):
  - proposal head: hidden^T via PE transposes, prop^T = W_prop^T @ hidden^T
  - span gathers (start/end/width rows) via indirect DMA
  - ragged mean-pool as a mask matmul: pooled^T = hidden^T @ mask01 * (1/w)
  - rep^T assembled feature-major; two big GEMMs with W stationary
  - type head + softmax + sensitivity head, outputs written transposed

DMA queue split: hidden + W_t1 stream on the SP (sync) HWDGE ring,
consts + W_s1 stream + outputs on the Activation ring, gathers on SWDGE.
"""

import sys
import contextlib

sys.path.insert(0, "/opt/trn_rl_repo")

import numpy as np
import concourse.bass as bass
import concourse.mybir as mybir
import concourse.tile as tile
from concourse import bacc
from concourse.bass import ts
from concourse.bass_utils import run_bass_kernel_spmd
from concourse.masks import make_identity

f32 = mybir.dt.float32
f32r = mybir.dt.float32r
i32 = mybir.dt.int32
AF = mybir.ActivationFunctionType
ALU = mybir.AluOpType
AX = mybir.AxisListType

B, S, H = 8, 2048, 768
N = 512  # spans
HW = 384  # width emb dim = H // 2
SPAN_DIM = H * 3 + HW  # 2688
N_BIO, N_TYPE, N_SENS = 3, 18, 4
P = 128
SB = S // P  # 16 s-blocks
HB = H // P  # 6 h-chunks
NJ = N // P  # 4 span-chunks
KREP = SPAN_DIM // P  # 21 rep k-chunks
WIDTH_VOCAB = 64


def _build_program():
    nc = bacc.Bacc("TRN2", target_bir_lowering=False, debug=False, num_devices=B)

    hid_d = nc.dram_tensor("hidden", [S, H], f32r, kind="ExternalInput").ap()
    gidx_d = nc.dram_tensor("gidx", [3, N], i32, kind="ExternalInput").ap()
    spanf_d = nc.dram_tensor("spanf", [3, N], f32, kind="ExternalInput").ap()
    wemb_d = nc.dram_tensor("width_emb", [WIDTH_VOCAB, HW], f32r, kind="ExternalInput").ap()
    wprop_d = nc.dram_tensor("W_prop", [H, N_BIO], f32r, kind="ExternalInput").ap()
    bprop_d = nc.dram_tensor("b_prop", [N_BIO, 1], f32, kind="ExternalInput").ap()
    wt1_d = nc.dram_tensor("W_t1", [SPAN_DIM, H], f32r, kind="ExternalInput").ap()
    bt1_d = nc.dram_tensor("b_t1", [HB, P], f32, kind="ExternalInput").ap()
    wt2_d = nc.dram_tensor("W_t2", [H, N_TYPE], f32r, kind="ExternalInput").ap()
    bt2_d = nc.dram_tensor("b_t2", [N_TYPE, 1], f32, kind="ExternalInput").ap()
    ws1_d = nc.dram_tensor("W_s1", [SPAN_DIM + N_TYPE, H], f32r, kind="ExternalInput").ap()
    bs1_d = nc.dram_tensor("b_s1", [HB, P], f32, kind="ExternalInput").ap()
    ws2_d = nc.dram_tensor("W_s2", [H, N_SENS], f32r, kind="ExternalInput").ap()
    bs2_d = nc.dram_tensor("b_s2", [N_SENS, 1], f32, kind="ExternalInput").ap()

    prop_o = nc.dram_tensor("prop_out", [S, N_BIO], f32, kind="ExternalOutput").ap()
    type_o = nc.dram_tensor("type_out", [N, N_TYPE], f32, kind="ExternalOutput").ap()
    sens_o = nc.dram_tensor("sens_out", [N, N_SENS], f32, kind="ExternalOutput").ap()

    with tile.TileContext(nc) as tc, contextlib.ExitStack() as ctx:
        consts = ctx.enter_context(tc.tile_pool(name="consts", bufs=1))
        hidp = ctx.enter_context(tc.tile_pool(name="hidp", bufs=1))
        rtp = ctx.enter_context(tc.tile_pool(name="rtp", bufs=1))
        outs = ctx.enter_context(tc.tile_pool(name="outs", bufs=1))

        # ---- early small inputs on the SP ring (before hidden) ----
        startb = consts.tile([P, N], f32)
        end1b = consts.tile([P, N], f32)
        recipb = consts.tile([P, N], f32)
        nc.sync.dma_start(startb[:], bass.AP(spanf_d.tensor, 0 * N, [[0, P], [1, N]]))
        nc.sync.dma_start(end1b[:], bass.AP(spanf_d.tensor, 1 * N, [[0, P], [1, N]]))
        nc.sync.dma_start(recipb[:], bass.AP(spanf_d.tensor, 2 * N, [[0, P], [1, N]]))

        # ---- hidden tiles stream in on SP ----
        hid_sb = []
        for k in range(SB):
            t = hidp.tile([P, H], f32r, name=f"hid{k}", tag=f"hid{k}")
            nc.sync.dma_start(t[:], hid_d[ts(k, P), :])
            hid_sb.append(t)

        # span indices load after hidden so the indirect gathers (which wait on
        # this tile) don't steal DMA bandwidth from the hidden stream
        idx_sb = consts.tile([P, 3, NJ], i32)
        nc.sync.dma_start(
            idx_sb[:, :, :], bass.AP(gidx_d.tensor, 0, [[1, P], [N, 3], [P, NJ]])
        )

        # ---- gpsimd: identity + iota, then gathers ----
        ident_f = consts.tile([P, P], f32)
        make_identity(nc, ident_f)
        ident = consts.tile([P, P], f32r)
        nc.vector.tensor_copy(ident[:], ident_f[:])
        iota_i = consts.tile([P, SB], i32)
        nc.gpsimd.iota(iota_i[:], pattern=[[P, SB]], base=0, channel_multiplier=1)
        iota_f = consts.tile([P, SB], f32)
        nc.vector.tensor_copy(iota_f[:], iota_i[:])

        # ---- remaining consts on the Activation ring ----
        bt1_sb = consts.tile([P, HB], f32)
        nc.scalar.dma_start(bt1_sb[:], bass.AP(bt1_d.tensor, 0, [[1, P], [P, HB]]))
        bs1_sb = consts.tile([P, HB], f32)
        nc.scalar.dma_start(bs1_sb[:], bass.AP(bs1_d.tensor, 0, [[1, P], [P, HB]]))
        bprop_sb = consts.tile([N_BIO, 1], f32)
        nc.scalar.dma_start(bprop_sb[:], bprop_d[:, :])
        bt2_sb = consts.tile([N_TYPE, 1], f32)
        nc.scalar.dma_start(bt2_sb[:], bt2_d[:, :])
        bs2_sb = consts.tile([N_SENS, 1], f32)
        nc.scalar.dma_start(bs2_sb[:], bs2_d[:, :])
        wprop_sb = consts.tile([P, HB, N_BIO], f32r)
        nc.scalar.dma_start(
            wprop_sb[:, :, :],
            bass.AP(wprop_d.tensor, 0, [[N_BIO, P], [N_BIO * P, HB], [1, N_BIO]]),
        )
        wt2_sb = consts.tile([P, HB, N_TYPE], f32r)
        nc.scalar.dma_start(
            wt2_sb[:, :, :],
            bass.AP(wt2_d.tensor, 0, [[N_TYPE, P], [N_TYPE * P, HB], [1, N_TYPE]]),
        )
        ws2_sb = consts.tile([P, HB, N_SENS], f32r)
        nc.scalar.dma_start(
            ws2_sb[:, :, :],
            bass.AP(ws2_d.tensor, 0, [[N_SENS, P], [N_SENS * P, HB], [1, N_SENS]]),
        )

        # ---- persistent rep^T tiles ----
        rt = [rtp.tile([P, N], f32r, name=f"rt{k}", tag=f"rt{k}") for k in range(KREP)]
        probsT = rtp.tile([N_TYPE, N], f32r, name="probsT", tag="probsT")

        # ---- persistent head-output tiles (tT and sT share slots) ----
        tT = [outs.tile([P, N], f32r, name=f"tT{m}", tag=f"hb{m}") for m in range(HB)]
        typeT_sb = outs.tile([N_TYPE, N], f32, name="typeT_sb", tag="typeT_sb")
        sensT_sb = outs.tile([N_SENS, N], f32, name="sensT_sb", tag="sensT_sb")
        probs = [
            outs.tile([P, N_TYPE], f32, name=f"probs{j}", tag=f"probs{j}")
            for j in range(NJ)
        ]

        # =========== Phase A: masks + pooled mask-matmul (starts as hidden streams in)
        with tc.tile_pool(name="maskp", bufs=1) as maskp, tc.tile_pool(
            name="psA", bufs=1, space="PSUM"
        ) as psA:
            gpsA = [
                psA.tile([P, N], f32, name=f"gpsA{m}", tag=f"gpsA{m}") for m in range(HB)
            ]
            m01s = []
            for k in range(SB):
                geS = maskp.tile([P, N], f32, name="geS", tag="geS", bufs=2)
                geE = maskp.tile([P, N], f32, name="geE", tag="geE", bufs=2)
                m01 = maskp.tile([P, N], f32r, name="m01", tag="m01", bufs=4)
                nc.vector.tensor_scalar(
                    out=geS[:], in0=startb[:], scalar1=iota_f[:, k : k + 1],
                    scalar2=None, op0=ALU.is_le,
                )
                nc.vector.tensor_scalar(
                    out=geE[:], in0=end1b[:], scalar1=iota_f[:, k : k + 1],
                    scalar2=None, op0=ALU.is_le,
                )
                nc.vector.tensor_tensor(out=m01[:], in0=geS[:], in1=geE[:], op=ALU.subtract)
                m01s.append(m01)
                for m in range(HB):
                    nc.tensor.matmul(
                        gpsA[m][:, :], hid_sb[k][:, ts(m, P)], m01[:],
                        start=(k == 0), stop=(k == SB - 1),
                    )
            for m in range(HB):
                nc.vector.tensor_tensor(
                    out=rt[2 * HB + m][:, :], in0=gpsA[m][:, :], in1=recipb[:], op=ALU.mult
                )

        # =========== Phase B: hidden^T + proposal, rep^T gathered parts
        with tc.tile_pool(name="gathp", bufs=1) as gathp, tc.tile_pool(
            name="htp", bufs=2
        ) as htp, tc.tile_pool(name="propsb", bufs=2) as propsbp, tc.tile_pool(
            name="psB", bufs=1, space="PSUM"
        ) as psB:
            sh, eh, wh = [], [], []
            for j in range(NJ):
                g = gathp.tile([P, H], f32r, name=f"sh{j}", tag="sh", bufs=2)
                nc.gpsimd.indirect_dma_start(
                    out=g[:], out_offset=None, in_=hid_d[:, :],
                    in_offset=bass.IndirectOffsetOnAxis(ap=idx_sb[:, 0, j : j + 1], axis=0),
                )
                sh.append(g)
                g = gathp.tile([P, H], f32r, name=f"eh{j}", tag="eh", bufs=2)
                nc.gpsimd.indirect_dma_start(
                    out=g[:], out_offset=None, in_=hid_d[:, :],
                    in_offset=bass.IndirectOffsetOnAxis(ap=idx_sb[:, 1, j : j + 1], axis=0),
                )
                eh.append(g)
                g = gathp.tile([P, HW], f32r, name=f"wh{j}", tag="wh", bufs=2)
                nc.gpsimd.indirect_dma_start(
                    out=g[:], out_offset=None, in_=wemb_d[:, :],
                    in_offset=bass.IndirectOffsetOnAxis(ap=idx_sb[:, 2, j : j + 1], axis=0),
                )
                wh.append(g)

            # hidden^T n-groups + proposal
            for n in range(NJ):
                hts = [
                    htp.tile([P, N], f32r, name=f"ht{h}", tag=f"ht{h}")
                    for h in range(HB)
                ]
                for kk in range(NJ):
                    k = n * NJ + kk
                    for h in range(HB):
                        ptr = psB.tile([P, P], f32r, name="ptr", tag="tr", bufs=6)
                        nc.tensor.transpose(ptr[:], hid_sb[k][:, ts(h, P)], ident[:])
                        nc.vector.tensor_copy(hts[h][:, ts(kk, P)], ptr[:])
                prop_ps = psB.tile([N_BIO, N], f32, name="prop_ps", tag="prop", bufs=2)
                for h in range(HB):
                    nc.tensor.matmul(
                        prop_ps[:, :], wprop_sb[:, h, :], hts[h][:, :],
                        start=(h == 0), stop=(h == HB - 1),
                    )
                psb = propsbp.tile([N_BIO, N], f32, name="psb", tag="psb")
                nc.scalar.activation(
                    psb[:, :], prop_ps[:, :], AF.Identity, bias=bprop_sb[:, 0:1], scale=1.0
                )
                nc.scalar.dma_start(
                    bass.AP(prop_o.tensor, n * N * N_BIO, [[1, N_BIO], [N_BIO, N]]),
                    psb[:, :],
                )

            # rep^T gathered parts: start (k 0-5), end (6-11), width (18-20)
            for j in range(NJ):
                for k in range(HB):
                    ptr = psB.tile([P, P], f32r, name="ptr", tag="tr", bufs=6)
                    nc.tensor.transpose(ptr[:], sh[j][:, ts(k, P)], ident[:])
                    nc.vector.tensor_copy(rt[k][:, ts(j, P)], ptr[:])
                for k in range(HB):
                    ptr = psB.tile([P, P], f32r, name="ptr", tag="tr", bufs=6)
                    nc.tensor.transpose(ptr[:], eh[j][:, ts(k, P)], ident[:])
                    nc.vector.tensor_copy(rt[HB + k][:, ts(j, P)], ptr[:])
                for k in range(HW // P):
                    ptr = psB.tile([P, P], f32r, name="ptr", tag="tr", bufs=6)
                    nc.tensor.transpose(ptr[:], wh[j][:, ts(k, P)], ident[:])
                    nc.vector.tensor_copy(rt[3 * HB + k][:, ts(j, P)], ptr[:])

        # =========== Phase C: GEMMs + heads
        with tc.tile_pool(name="softp", bufs=1) as softp, tc.tile_pool(
            name="wsp", bufs=1
        ) as wsp, tc.tile_pool(name="psmm", bufs=1, space="PSUM") as psmm, tc.tile_pool(
            name="pshead", bufs=1, space="PSUM"
        ) as pshead:
            gps = [
                psmm.tile([P, N], f32, name=f"gps{m}", tag=f"gps{m}") for m in range(HB)
            ]

            # prefetch the entire W_s1 stream on the (idle) SWDGE queue with a
            # deep buffer so GEMM2 is never DMA-paced
            ws1_tiles = []
            for k in range(KREP):
                ws = wsp.tile([P, H], f32r, name="ws1s", tag="ws1s", bufs=12)
                nc.gpsimd.dma_start(ws[:], ws1_d[ts(k, P), :])
                ws1_tiles.append(ws)
            ws_last = wsp.tile([P, H], f32r, name="ws1s", tag="ws1s", bufs=12)
            nc.gpsimd.dma_start(ws_last[:N_TYPE, :], ws1_d[SPAN_DIM : SPAN_DIM + N_TYPE, :])

            # GEMM1: t^T = gelu(W_t1^T @ rep^T + b_t1)   (weights stream on SP ring)
            for k in range(KREP):
                wt = wsp.tile([P, H], f32r, name="wt1s", tag="wt1s", bufs=4)
                nc.sync.dma_start(wt[:], wt1_d[ts(k, P), :])
                for m in range(HB):
                    nc.tensor.matmul(
                        gps[m][:, :], wt[:, ts(m, P)], rt[k][:, :],
                        start=(k == 0), stop=(k == KREP - 1),
                    )
            for m in range(HB):
                nc.scalar.activation(
                    tT[m][:, :], gps[m][:, :], AF.Gelu, bias=bt1_sb[:, m : m + 1], scale=1.0
                )

            # type head
            typeT_ps = pshead.tile([N_TYPE, N], f32, name="typeT_ps", tag="headT")
            for k in range(HB):
                nc.tensor.matmul(
                    typeT_ps[:, :], wt2_sb[:, k, :], tT[k][:, :],
                    start=(k == 0), stop=(k == HB - 1),
                )
            nc.scalar.activation(
                typeT_sb[:, :], typeT_ps[:, :], AF.Identity, bias=bt2_sb[:, 0:1], scale=1.0
            )
            nc.scalar.dma_start(
                bass.AP(type_o.tensor, 0, [[1, N_TYPE], [N_TYPE, N]]), typeT_sb[:, :]
            )

            # softmax (span-major)
            for j in range(NJ):
                ttr = pshead.tile([P, N_TYPE], f32, name="ttr", tag="soft")
                nc.tensor.transpose(
                    ttr[:], typeT_sb[:, ts(j, P)], ident_f[:N_TYPE, :N_TYPE]
                )
                mx = softp.tile([P, 1], f32, name="mx", tag="mx", bufs=2)
                nc.vector.tensor_reduce(
                    out=mx[:], in_=ttr[:], axis=AX.X, op=ALU.max, negate=True
                )
                ex = softp.tile([P, N_TYPE], f32, name="ex", tag="ex", bufs=2)
                sm = softp.tile([P, 1], f32, name="sm", tag="sm", bufs=2)
                nc.scalar.activation(
                    ex[:], ttr[:], AF.Exp, bias=mx[:, 0:1], scale=1.0, accum_out=sm[:, 0:1]
                )
                rc = softp.tile([P, 1], f32, name="rc", tag="rc", bufs=2)
                nc.vector.reciprocal(rc[:], sm[:])
                nc.vector.tensor_scalar(
                    out=probs[j][:], in0=ex[:], scalar1=rc[:, 0:1], scalar2=None, op0=ALU.mult
                )

            # GEMM2 part A: rep^T rows (weights prefetched above)
            for k in range(KREP):
                ws = ws1_tiles[k]
                for m in range(HB):
                    nc.tensor.matmul(
                        gps[m][:, :], ws[:, ts(m, P)], rt[k][:, :],
                        start=(k == 0), stop=False,
                    )

            # probs^T chunk + final GEMM2 accumulation
            for j in range(NJ):
                ptr2 = pshead.tile([N_TYPE, P], f32, name="ptr2", tag="soft")
                nc.tensor.transpose(ptr2[:], probs[j][:, :], ident_f[:, :])
                nc.vector.tensor_copy(probsT[:, ts(j, P)], ptr2[:])
            for m in range(HB):
                nc.tensor.matmul(
                    gps[m][:, :], ws_last[:N_TYPE, ts(m, P)], probsT[:, :],
                    start=False, stop=True,
                )
            sT = [outs.tile([P, N], f32r, name=f"sT{m}", tag=f"hb{m}") for m in range(HB)]
            for m in range(HB):
                nc.scalar.activation(
                    sT[m][:, :], gps[m][:, :], AF.Gelu, bias=bs1_sb[:, m : m + 1], scale=1.0
                )

            # sensitivity head
            sensT_ps = pshead.tile([N_SENS, N], f32, name="sensT_ps", tag="headT")
            for k in range(HB):
                nc.tensor.matmul(
                    sensT_ps[:, :], ws2_sb[:, k, :], sT[k][:, :],
                    start=(k == 0), stop=(k == HB - 1),
                )
            nc.scalar.activation(
                sensT_sb[:, :], sensT_ps[:, :], AF.Identity, bias=bs2_sb[:, 0:1], scale=1.0
            )
            nc.scalar.dma_start(
                bass.AP(sens_o.tensor, 0, [[1, N_SENS], [N_SENS, N]]), sensT_sb[:, :]
            )

    nc.compile()
    return nc


_NC_CACHE = None


def _get_program():
    global _NC_CACHE
    if _NC_CACHE is None:
        _NC_CACHE = _build_program()
    return _NC_CACHE


def _make_in_maps(inputs):
    hidden = np.asarray(inputs["hidden"], dtype=np.float32)
    spans = np.asarray(inputs["candidate_spans"], dtype=np.int64)
    width_emb = np.ascontiguousarray(np.asarray(inputs["width_emb"], np.float32))
    W_prop = np.ascontiguousarray(np.asarray(inputs["W_prop"], np.float32))
    b_prop = np.asarray(inputs["b_prop"], np.float32).reshape(N_BIO, 1)
    W_t1 = np.ascontiguousarray(np.asarray(inputs["W_t1"], np.float32))
    b_t1 = np.asarray(inputs["b_t1"], np.float32).reshape(HB, P)
    W_t2 = np.ascontiguousarray(np.asarray(inputs["W_t2"], np.float32))
    b_t2 = np.asarray(inputs["b_t2"], np.float32).reshape(N_TYPE, 1)
    W_s1 = np.ascontiguousarray(np.asarray(inputs["W_s1"], np.float32))
    b_s1 = np.asarray(inputs["b_s1"], np.float32).reshape(HB, P)
    W_s2 = np.ascontiguousarray(np.asarray(inputs["W_s2"], np.float32))
    b_s2 = np.asarray(inputs["b_s2"], np.float32).reshape(N_SENS, 1)

    starts = spans[..., 0]
    ends = spans[..., 1]
    widths = ends - starts + 1
    in_maps = []
    for b in range(B):
        gidx = np.stack(
            [starts[b], ends[b], np.minimum(widths[b], WIDTH_VOCAB - 1)]
        ).astype(np.int32)
        spanf = np.stack(
            [
                starts[b].astype(np.float32),
                (ends[b] + 1).astype(np.float32),
                (1.0 / widths[b]).astype(np.float32),
            ]
        ).astype(np.float32)
        in_maps.append(
            {
                "hidden": np.ascontiguousarray(hidden[b]),
                "gidx": gidx,
                "spanf": spanf,
                "width_emb": width_emb,
                "W_prop": W_prop,
                "b_prop": b_prop,
                "W_t1": W_t1,
                "b_t1": b_t1,
                "W_t2": W_t2,
                "b_t2": b_t2,
                "W_s1": W_s1,
                "b_s1": b_s1,
                "W_s2": W_s2,
                "b_s2": b_s2,
            }
        )
    return in_maps


def run(inputs, trace=False):
    nc = _get_program()
    in_maps = _make_in_maps(inputs)
    res = run_bass_kernel_spmd(nc, in_maps, core_ids=list(range(B)), trace=trace)
    prop = np.stack([res.results[b]["prop_out"] for b in range(B)])
    typ = np.stack([res.results[b]["type_out"] for b in range(B)])
    sens = np.stack([res.results[b]["sens_out"] for b in range(B)])
    return (prop, typ, sens), res


def kernel(**inputs):
    out, _ = run(inputs, trace=False)
    return out


# revision 6
# speedup vs baseline: 1.0306x; 1.0306x over previous
"""Trainium2 Bass kernel for nn_MultiHeadPiiModel (segment_reduce).

Data-parallel over batch: 8 NeuronCores, one batch element each.
Per-core pipeline (all matmuls in fp32r):
  - proposal head: hidden^T via PE transposes, prop^T = W_prop^T @ hidden^T
  - span gathers (start/end/width rows) via indirect DMA
  - ragged mean-pool as a mask matmul: pooled^T = hidden^T @ mask01 * (1/w)
  - rep^T assembled feature-major; two big GEMMs with W stationary
  - type head + softmax + sensitivity head, outputs written transposed

DMA queue split: hidden + W_t1 stream on the SP (sync) HWDGE ring,
consts + W_s1 stream + outputs on the Activation ring, gathers on SWDGE.
"""

import sys
import contextlib

sys.path.insert(0, "/opt/trn_rl_repo")

import numpy as np
import concourse.bass as bass
import concourse.mybir as mybir
import concourse.tile as tile
from concourse import bacc
from concourse.bass import ts
from concourse.bass_utils import run_bass_kernel_spmd
from concourse.masks import make_identity

f32 = mybir.dt.float32
f32r = mybir.dt.float32r
i32 = mybir.dt.int32
AF = mybir.ActivationFunctionType
ALU = mybir.AluOpType
AX = mybir.AxisListType

B, S, H = 8, 2048, 768
N = 512  # spans
HW = 384  # width emb dim = H // 2
SPAN_DIM = H * 3 + HW  # 2688
N_BIO, N_TYPE, N_SENS = 3, 18, 4
P = 128
SB = S // P  # 16 s-blocks
HB = H // P  # 6 h-chunks
NJ = N // P  # 4 span-chunks
KREP = SPAN_DIM // P  # 21 rep k-chunks
WIDTH_VOCAB = 64


def _build_program():
    nc = bacc.Bacc("TRN2", target_bir_lowering=False, debug=False, num_devices=B)

    hid_d = nc.dram_tensor("hidden", [S, H], f32r, kind="ExternalInput").ap()
    gidx_d = nc.dram_tensor("gidx", [3, N], i32, kind="ExternalInput").ap()
    spanf_d = nc.dram_tensor("spanf", [3, N], f32, kind="ExternalInput").ap()
    wemb_d = nc.dram_tensor("width_emb", [WIDTH_VOCAB, HW], f32r, kind="ExternalInput").ap()
    wprop_d = nc.dram_tensor("W_prop", [H, N_BIO], f32r, kind="ExternalInput").ap()
    bprop_d = nc.dram_tensor("b_prop", [N_BIO, 1], f32, kind="ExternalInput").ap()
    wt1_d = nc.dram_tensor("W_t1", [SPAN_DIM, H], f32r, kind="ExternalInput").ap()
    bt1_d = nc.dram_tensor("b_t1", [HB, P], f32, kind="ExternalInput").ap()
    wt2_d = nc.dram_tensor("W_t2", [H, N_TYPE], f32r, kind="ExternalInput").ap()
    bt2_d = nc.dram_tensor("b_t2", [N_TYPE, 1], f32, kind="ExternalInput").ap()
    ws1_d = nc.dram_tensor("W_s1", [SPAN_DIM + N_TYPE, H], f32r, kind="ExternalInput").ap()
    bs1_d = nc.dram_tensor("b_s1", [HB, P], f32, kind="ExternalInput").ap()
    ws2_d = nc.dram_tensor("W_s2", [H, N_SENS], f32r, kind="ExternalInput").ap()
    bs2_d = nc.dram_tensor("b_s2", [N_SENS, 1], f32, kind="ExternalInput").ap()

    prop_o = nc.dram_tensor("prop_out", [S, N_BIO], f32, kind="ExternalOutput").ap()
    type_o = nc.dram_tensor("type_out", [N, N_TYPE], f32, kind="ExternalOutput").ap()
    sens_o = nc.dram_tensor("sens_out", [N, N_SENS], f32, kind="ExternalOutput").ap()

    with tile.TileContext(nc) as tc, contextlib.ExitStack() as ctx:
        consts = ctx.enter_context(tc.tile_pool(name="consts", bufs=1))
        hidp = ctx.enter_context(tc.tile_pool(name="hidp", bufs=1))
        rtp = ctx.enter_context(tc.tile_pool(name="rtp", bufs=1))
        outs = ctx.enter_context(tc.tile_pool(name="outs", bufs=1))

        # ---- early small inputs on the SP ring (before hidden) ----
        startb = consts.tile([P, N], f32)
        end1b = consts.tile([P, N], f32)
        recipb = consts.tile([P, N], f32)
        nc.sync.dma_start(startb[:], bass.AP(spanf_d.tensor, 0 * N, [[0, P], [1, N]]))
        nc.sync.dma_start(end1b[:], bass.AP(spanf_d.tensor, 1 * N, [[0, P], [1, N]]))
        nc.sync.dma_start(recipb[:], bass.AP(spanf_d.tensor, 2 * N, [[0, P], [1, N]]))

        # ---- hidden tiles stream in on SP ----
        hid_sb = []
        for k in range(SB):
            t = hidp.tile([P, H], f32r, name=f"hid{k}", tag=f"hid{k}")
            nc.sync.dma_start(t[:], hid_d[ts(k, P), :])
            hid_sb.append(t)

        # span indices load after hidden so the indirect gathers (which wait on
        # this tile) don't steal DMA bandwidth from the hidden stream
        idx_sb = consts.tile([P, 3, NJ], i32)
        nc.sync.dma_start(
            idx_sb[:, :, :], bass.AP(gidx_d.tensor, 0, [[1, P], [N, 3], [P, NJ]])
        )

        # ---- gpsimd: identity + iota, then gathers ----
        ident_f = consts.tile([P, P], f32)
        make_identity(nc, ident_f)
        ident = consts.tile([P, P], f32r)
        nc.vector.tensor_copy(ident[:], ident_f[:])
        iota_i = consts.tile([P, SB], i32)
        nc.gpsimd.iota(iota_i[:], pattern=[[P, SB]], base=0, channel_multiplier=1)
        iota_f = consts.tile([P, SB], f32)
        nc.vector.tensor_copy(iota_f[:], iota_i[:])

        # ---- remaining consts on the Activation ring ----
        bt1_sb = consts.tile([P, HB], f32)
        nc.scalar.dma_start(bt1_sb[:], bass.AP(bt1_d.tensor, 0, [[1, P], [P, HB]]))
        bs1_sb = consts.tile([P, HB], f32)
        nc.scalar.dma_start(bs1_sb[:], bass.AP(bs1_d.tensor, 0, [[1, P], [P, HB]]))
        bprop_sb = consts.tile([N_BIO, 1], f32)
        nc.scalar.dma_start(bprop_sb[:], bprop_d[:, :])
        bt2_sb = consts.tile([N_TYPE, 1], f32)
        nc.scalar.dma_start(bt2_sb[:], bt2_d[:, :])
        bs2_sb = consts.tile([N_SENS, 1], f32)
        nc.scalar.dma_start(bs2_sb[:], bs2_d[:, :])
        wprop_sb = consts.tile([P, HB, N_BIO], f32r)
        nc.scalar.dma_start(
            wprop_sb[:, :, :],
            bass.AP(wprop_d.tensor, 0, [[N_BIO, P], [N_BIO * P, HB], [1, N_BIO]]),
        )
        wt2_sb = consts.tile([P, HB, N_TYPE], f32r)
        nc.scalar.dma_start(
            wt2_sb[:, :, :],
            bass.AP(wt2_d.tensor, 0, [[N_TYPE, P], [N_TYPE * P, HB], [1, N_TYPE]]),
        )
        ws2_sb = consts.tile([P, HB, N_SENS], f32r)
        nc.scalar.dma_start(
            ws2_sb[:, :, :],
            bass.AP(ws2_d.tensor, 0, [[N_SENS, P], [N_SENS * P, HB], [1, N_SENS]]),
        )

        # ---- persistent rep^T tiles ----
        rt = [rtp.tile([P, N], f32r, name=f"rt{k}", tag=f"rt{k}") for k in range(KREP)]
        probsT = rtp.tile([N_TYPE, N], f32r, name="probsT", tag="probsT")

        # ---- persistent head-output tiles (tT and sT share slots) ----
        tT = [outs.tile([P, N], f32r, name=f"tT{m}", tag=f"hb{m}") for m in range(HB)]
        typeT_sb = outs.tile([N_TYPE, N], f32, name="typeT_sb", tag="typeT_sb")
        sensT_sb = outs.tile([N_SENS, N], f32, name="sensT_sb", tag="sensT_sb")
        probs = [
            outs.tile([P, N_TYPE], f32, name=f"probs{j}", tag=f"probs{j}")
            for j in range(NJ)
        ]

        # =========== Phase A: masks + pooled mask-matmul (starts as hidden streams in)
        with tc.tile_pool(name="maskp", bufs=1) as maskp, tc.tile_pool(
            name="psA", bufs=1, space="PSUM"
        ) as psA:
            gpsA = [
                psA.tile([P, N], f32, name=f"gpsA{m}", tag=f"gpsA{m}") for m in range(HB)
            ]
            m01s = []
            for k in range(SB):
                geS = maskp.tile([P, N], f32, name="geS", tag="geS", bufs=2)
                geE = maskp.tile([P, N], f32, name="geE", tag="geE", bufs=2)
                m01 = maskp.tile([P, N], f32r, name="m01", tag="m01", bufs=4)
                nc.vector.tensor_scalar(
                    out=geS[:], in0=startb[:], scalar1=iota_f[:, k : k + 1],
                    scalar2=None, op0=ALU.is_le,
                )
                nc.vector.tensor_scalar(
                    out=geE[:], in0=end1b[:], scalar1=iota_f[:, k : k + 1],
                    scalar2=None, op0=ALU.is_le,
                )
                nc.vector.tensor_tensor(out=m01[:], in0=geS[:], in1=geE[:], op=ALU.subtract)
                m01s.append(m01)
                for m in range(HB):
                    nc.tensor.matmul(
                        gpsA[m][:, :], hid_sb[k][:, ts(m, P)], m01[:],
                        start=(k == 0), stop=(k == SB - 1),
                    )
            for m in range(HB):
                nc.vector.tensor_tensor(
                    out=rt[2 * HB + m][:, :], in0=gpsA[m][:, :], in1=recipb[:], op=ALU.mult
                )

        # =========== Phase B: hidden^T + proposal, rep^T gathered parts
        with tc.tile_pool(name="gathp", bufs=1) as gathp, tc.tile_pool(
            name="htp", bufs=2
        ) as htp, tc.tile_pool(name="propsb", bufs=2) as propsbp, tc.tile_pool(
            name="psB", bufs=1, space="PSUM"
        ) as psB:
            sh, eh, wh = [], [], []
            for j in range(NJ):
                g = gathp.tile([P, H], f32r, name=f"sh{j}", tag="sh", bufs=2)
                nc.gpsimd.indirect_dma_start(
                    out=g[:], out_offset=None, in_=hid_d[:, :],
                    in_offset=bass.IndirectOffsetOnAxis(ap=idx_sb[:, 0, j : j + 1], axis=0),
                )
                sh.append(g)
                g = gathp.tile([P, H], f32r, name=f"eh{j}", tag="eh", bufs=2)
                nc.gpsimd.indirect_dma_start(
                    out=g[:], out_offset=None, in_=hid_d[:, :],
                    in_offset=bass.IndirectOffsetOnAxis(ap=idx_sb[:, 1, j : j + 1], axis=0),
                )
                eh.append(g)
                g = gathp.tile([P, HW], f32r, name=f"wh{j}", tag="wh", bufs=2)
                nc.gpsimd.indirect_dma_start(
                    out=g[:], out_offset=None, in_=wemb_d[:, :],
                    in_offset=bass.IndirectOffsetOnAxis(ap=idx_sb[:, 2, j : j + 1], axis=0),
                )
                wh.append(g)

            # hidden^T n-groups + proposal
            for n in range(NJ):
                hts = [
                    htp.tile([P, N], f32r, name=f"ht{h}", tag=f"ht{h}")
                    for h in range(HB)
                ]
                for kk in range(NJ):
                    k = n * NJ + kk
                    for h in range(HB):
                        ptr = psB.tile([P, P], f32r, name="ptr", tag="tr", bufs=6)
                        nc.tensor.transpose(ptr[:], hid_sb[k][:, ts(h, P)], ident[:])
                        nc.vector.tensor_copy(hts[h][:, ts(kk, P)], ptr[:])
                prop_ps = psB.tile([N_BIO, N], f32, name="prop_ps", tag="prop", bufs=2)
                for h in range(HB):
                    nc.tensor.matmul(
                        prop_ps[:, :], wprop_sb[:, h, :], hts[h][:, :],
                        start=(h == 0), stop=(h == HB - 1),
                    )
                psb = propsbp.tile([N_BIO, N], f32, name="psb", tag="psb")
                nc.scalar.activation(
                    psb[:, :], prop_ps[:, :], AF.Identity, bias=bprop_sb[:, 0:1], scale=1.0
                )
                nc.scalar.dma_start(
                    bass.AP(prop_o.tensor, n * N * N_BIO, [[1, N_BIO], [N_BIO, N]]),
                    psb[:, :],
                )

            # rep^T gathered parts: start (k 0-5), end (6-11), width (18-20)
            for j in range(NJ):
                for k in range(HB):
                    ptr = psB.tile([P, P], f32r, name="ptr", tag="tr", bufs=6)
                    nc.tensor.transpose(ptr[:], sh[j][:, ts(k, P)], ident[:])
                    nc.vector.tensor_copy(rt[k][:, ts(j, P)], ptr[:])
                for k in range(HB):
                    ptr = psB.tile([P, P], f32r, name="ptr", tag="tr", bufs=6)
                    nc.tensor.transpose(ptr[:], eh[j][:, ts(k, P)], ident[:])
                    nc.vector.tensor_copy(rt[HB + k][:, ts(j, P)], ptr[:])
                for k in range(HW // P):
                    ptr = psB.tile([P, P], f32r, name="ptr", tag="tr", bufs=6)
                    nc.tensor.transpose(ptr[:], wh[j][:, ts(k, P)], ident[:])
                    nc.vector.tensor_copy(rt[3 * HB + k][:, ts(j, P)], ptr[:])

        # =========== Phase C: GEMMs + heads
        with tc.tile_pool(name="softp", bufs=1) as softp, tc.tile_pool(
            name="wsp", bufs=1
        ) as wsp, tc.tile_pool(name="psmm", bufs=1, space="PSUM") as psmm, tc.tile_pool(
            name="pshead", bufs=1, space="PSUM"
        ) as pshead:
            gps = [
                psmm.tile([P, N], f32, name=f"gps{m}", tag=f"gps{m}") for m in range(HB)
            ]

            # GEMM1: t^T = gelu(W_t1^T @ rep^T + b_t1)   (weights stream on SP ring)
            for k in range(KREP):
                wt = wsp.tile([P, H], f32r, name="wt1s", tag="wt1s", bufs=4)
                nc.sync.dma_start(wt[:], wt1_d[ts(k, P), :])
                for m in range(HB):
                    nc.tensor.matmul(
                        gps[m][:, :], wt[:, ts(m, P)], rt[k][:, :],
                        start=(k == 0), stop=(k == KREP - 1),
                    )
            for m in range(HB):
                nc.scalar.activation(
                    tT[m][:, :], gps[m][:, :], AF.Gelu, bias=bt1_sb[:, m : m + 1], scale=1.0
                )

            # type head
            typeT_ps = pshead.tile([N_TYPE, N], f32, name="typeT_ps", tag="headT")
            for k in range(HB):
                nc.tensor.matmul(
                    typeT_ps[:, :], wt2_sb[:, k, :], tT[k][:, :],
                    start=(k == 0), stop=(k == HB - 1),
                )
            nc.scalar.activation(
                typeT_sb[:, :], typeT_ps[:, :], AF.Identity, bias=bt2_sb[:, 0:1], scale=1.0
            )
            nc.scalar.dma_start(
                bass.AP(type_o.tensor, 0, [[1, N_TYPE], [N_TYPE, N]]), typeT_sb[:, :]
            )

            # softmax (span-major)
            for j in range(NJ):
                ttr = pshead.tile([P, N_TYPE], f32, name="ttr", tag="soft")
                nc.tensor.transpose(
                    ttr[:], typeT_sb[:, ts(j, P)], ident_f[:N_TYPE, :N_TYPE]
                )
                mx = softp.tile([P, 1], f32, name="mx", tag="mx", bufs=2)
                nc.vector.tensor_reduce(
                    out=mx[:], in_=ttr[:], axis=AX.X, op=ALU.max, negate=True
                )
                ex = softp.tile([P, N_TYPE], f32, name="ex", tag="ex", bufs=2)
                sm = softp.tile([P, 1], f32, name="sm", tag="sm", bufs=2)
                nc.scalar.activation(
                    ex[:], ttr[:], AF.Exp, bias=mx[:, 0:1], scale=1.0, accum_out=sm[:, 0:1]
                )
                rc = softp.tile([P, 1], f32, name="rc", tag="rc", bufs=2)
                nc.vector.reciprocal(rc[:], sm[:])
                nc.vector.tensor_scalar(
                    out=probs[j][:], in0=ex[:], scalar1=rc[:, 0:1], scalar2=None, op0=ALU.mult
                )

            # GEMM2 part A: rep^T rows (weights stream on SP ring after W_t1)
            for k in range(KREP):
                ws = wsp.tile([P, H], f32r, name="ws1s", tag="ws1s", bufs=8)
                nc.sync.dma_start(ws[:], ws1_d[ts(k, P), :])
                for m in range(HB):
                    nc.tensor.matmul(
                        gps[m][:, :], ws[:, ts(m, P)], rt[k][:, :],
                        start=(k == 0), stop=False,
                    )

            # probs^T chunk + final GEMM2 accumulation
            for j in range(NJ):
                ptr2 = pshead.tile([N_TYPE, P], f32, name="ptr2", tag="soft")
                nc.tensor.transpose(ptr2[:], probs[j][:, :], ident_f[:, :])
                nc.vector.tensor_copy(probsT[:, ts(j, P)], ptr2[:])
            ws_last = wsp.tile([P, H], f32r, name="ws1s", tag="ws1s", bufs=8)
            nc.sync.dma_start(ws_last[:N_TYPE, :], ws1_d[SPAN_DIM : SPAN_DIM + N_TYPE, :])
            for m in range(HB):
                nc.tensor.matmul(
                    gps[m][:, :], ws_last[:N_TYPE, ts(m, P)], probsT[:, :],
                    start=False, stop=True,
                )
            sT = [outs.tile([P, N], f32r, name=f"sT{m}", tag=f"hb{m}") for m in range(HB)]
            for m in range(HB):
                nc.scalar.activation(
                    sT[m][:, :], gps[m][:, :], AF.Gelu, bias=bs1_sb[:, m : m + 1], scale=1.0
                )

            # sensitivity head
            sensT_ps = pshead.tile([N_SENS, N], f32, name="sensT_ps", tag="headT")
            for k in range(HB):
                nc.tensor.matmul(
                    sensT_ps[:, :], ws2_sb[:, k, :], sT[k][:, :],
                    start=(k == 0), stop=(k == HB - 1),
                )
            nc.scalar.activation(
                sensT_sb[:, :], sensT_ps[:, :], AF.Identity, bias=bs2_sb[:, 0:1], scale=1.0
            )
            nc.scalar.dma_start(
                bass.AP(sens_o.tensor, 0, [[1, N_SENS], [N_SENS, N]]), sensT_sb[:, :]
            )

    nc.compile()
    return nc


_NC_CACHE = None


def _get_program():
    global _NC_CACHE
    if _NC_CACHE is None:
        _NC_CACHE = _build_program()
    return _NC_CACHE


def _make_in_maps(inputs):
    hidden = np.asarray(inputs["hidden"], dtype=np.float32)
    spans = np.asarray(inputs["candidate_spans"], dtype=np.int64)
    width_emb = np.ascontiguousarray(np.asarray(inputs["width_emb"], np.float32))
    W_prop = np.ascontiguousarray(np.asarray(inputs["W_prop"], np.float32))
    b_prop = np.asarray(inputs["b_prop"], np.float32).reshape(N_BIO, 1)
    W_t1 = np.ascontiguousarray(np.asarray(inputs["W_t1"], np.float32))
    b_t1 = np.asarray(inputs["b_t1"], np.float32).reshape(HB, P)
    W_t2 = np.ascontiguousarray(np.asarray(inputs["W_t2"], np.float32))
    b_t2 = np.asarray(inputs["b_t2"], np.float32).reshape(N_TYPE, 1)
    W_s1 = np.ascontiguousarray(np.asarray(inputs["W_s1"], np.float32))
    b_s1 = np.asarray(inputs["b_s1"], np.float32).reshape(HB, P)
    W_s2 = np.ascontiguousarray(np.asarray(inputs["W_s2"], np.float32))
    b_s2 = np.asarray(inputs["b_s2"], np.float32).reshape(N_SENS, 1)

    starts = spans[..., 0]
    ends = spans[..., 1]
    widths = ends - starts + 1
    in_maps = []
    for b in range(B):
        gidx = np.stack(
            [starts[b], ends[b], np.minimum(widths[b], WIDTH_VOCAB - 1)]
        ).astype(np.int32)
        spanf = np.stack(
            [
                starts[b].astype(np.float32),
                (ends[b] + 1).astype(np.float32),
                (1.0 / widths[b]).astype(np.float32),
            ]
        ).astype(np.float32)
        in_maps.append(
            {
                "hidden": np.ascontiguousarray(hidden[b]),
                "gidx": gidx,
                "spanf": spanf,
                "width_emb": width_emb,
                "W_prop": W_prop,
                "b_prop": b_prop,
                "W_t1": W_t1,
                "b_t1": b_t1,
                "W_t2": W_t2,
                "b_t2": b_t2,
                "W_s1": W_s1,
                "b_s1": b_s1,
                "W_s2": W_s2,
                "b_s2": b_s2,
            }
        )
    return in_maps


def run(inputs, trace=False):
    nc = _get_program()
    in_maps = _make_in_maps(inputs)
    res = run_bass_kernel_spmd(nc, in_maps, core_ids=list(range(B)), trace=trace)
    prop = np.stack([res.results[b]["prop_out"] for b in range(B)])
    typ = np.stack([res.results[b]["type_out"] for b in range(B)])
    sens = np.stack([res.results[b]["sens_out"] for b in range(B)])
    return (prop, typ, sens), res


def kernel(**inputs):
    out, _ = run(inputs, trace=False)
    return out


# revision 7
# speedup vs baseline: 1.3492x; 1.3092x over previous
"""Trainium2 Bass kernel for nn_MultiHeadPiiModel (segment_reduce).

Data-parallel over batch: 8 NeuronCores, one batch element each.
Per-core pipeline (all matmuls in fp32r):
  - proposal head: hidden^T via PE transposes, prop^T = W_prop^T @ hidden^T
  - span gathers (start/end/width rows) via indirect DMA
  - ragged mean-pool as a mask matmul: pooled^T = hidden^T @ mask01 * (1/w)
  - rep^T assembled feature-major; two big GEMMs with W stationary
  - type head + softmax + sensitivity head, outputs written transposed

DMA queue split: hidden + W_t1 stream on the SP (sync) HWDGE ring,
consts + W_s1 stream + outputs on the Activation ring, gathers on SWDGE.
"""

import sys
import contextlib

sys.path.insert(0, "/opt/trn_rl_repo")

import numpy as np
import concourse.bass as bass
import concourse.mybir as mybir
import concourse.tile as tile
from concourse import bacc
from concourse.bass import ts
from concourse.bass_utils import run_bass_kernel_spmd
from concourse.masks import make_identity

f32 = mybir.dt.float32
f32r = mybir.dt.float32r
i32 = mybir.dt.int32
AF = mybir.ActivationFunctionType
ALU = mybir.AluOpType
AX = mybir.AxisListType

B, S, H = 8, 2048, 768
N = 512  # spans
HW = 384  # width emb dim = H // 2
SPAN_DIM = H * 3 + HW  # 2688
N_BIO, N_TYPE, N_SENS = 3, 18, 4
P = 128
SB = S // P  # 16 s-blocks
HB = H // P  # 6 h-chunks
NJ = N // P  # 4 span-chunks
KREP = SPAN_DIM // P  # 21 rep k-chunks
WIDTH_VOCAB = 64


def _build_program():
    nc = bacc.Bacc("TRN2", target_bir_lowering=False, debug=False, num_devices=B)

    hid_d = nc.dram_tensor("hidden", [S, H], f32r, kind="ExternalInput").ap()
    gidx_d = nc.dram_tensor("gidx", [3, N], i32, kind="ExternalInput").ap()
    spanf_d = nc.dram_tensor("spanf", [3, N], f32, kind="ExternalInput").ap()
    wemb_d = nc.dram_tensor("width_emb", [WIDTH_VOCAB, HW], f32r, kind="ExternalInput").ap()
    wprop_d = nc.dram_tensor("W_prop", [H, N_BIO], f32r, kind="ExternalInput").ap()
    bprop_d = nc.dram_tensor("b_prop", [N_BIO, 1], f32, kind="ExternalInput").ap()
    wt1_d = nc.dram_tensor("W_t1", [SPAN_DIM, H], f32r, kind="ExternalInput").ap()
    bt1_d = nc.dram_tensor("b_t1", [HB, P], f32, kind="ExternalInput").ap()
    wt2_d = nc.dram_tensor("W_t2", [H, N_TYPE], f32r, kind="ExternalInput").ap()
    bt2_d = nc.dram_tensor("b_t2", [N_TYPE, 1], f32, kind="ExternalInput").ap()
    ws1_d = nc.dram_tensor("W_s1", [SPAN_DIM + N_TYPE, H], f32r, kind="ExternalInput").ap()
    bs1_d = nc.dram_tensor("b_s1", [HB, P], f32, kind="ExternalInput").ap()
    ws2_d = nc.dram_tensor("W_s2", [H, N_SENS], f32r, kind="ExternalInput").ap()
    bs2_d = nc.dram_tensor("b_s2", [N_SENS, 1], f32, kind="ExternalInput").ap()

    prop_o = nc.dram_tensor("prop_out", [S, N_BIO], f32, kind="ExternalOutput").ap()
    type_o = nc.dram_tensor("type_out", [N, N_TYPE], f32, kind="ExternalOutput").ap()
    sens_o = nc.dram_tensor("sens_out", [N, N_SENS], f32, kind="ExternalOutput").ap()

    with tile.TileContext(nc) as tc, contextlib.ExitStack() as ctx:
        consts = ctx.enter_context(tc.tile_pool(name="consts", bufs=1))
        hidp = ctx.enter_context(tc.tile_pool(name="hidp", bufs=1))
        rtp = ctx.enter_context(tc.tile_pool(name="rtp", bufs=1))
        outs = ctx.enter_context(tc.tile_pool(name="outs", bufs=1))

        # ---- early small inputs on the SP ring (before hidden) ----
        idx_sb = consts.tile([P, 3, NJ], i32)
        nc.sync.dma_start(
            idx_sb[:, :, :], bass.AP(gidx_d.tensor, 0, [[1, P], [N, 3], [P, NJ]])
        )
        startb = consts.tile([P, N], f32)
        end1b = consts.tile([P, N], f32)
        recipb = consts.tile([P, N], f32)
        nc.sync.dma_start(startb[:], bass.AP(spanf_d.tensor, 0 * N, [[0, P], [1, N]]))
        nc.sync.dma_start(end1b[:], bass.AP(spanf_d.tensor, 1 * N, [[0, P], [1, N]]))
        nc.sync.dma_start(recipb[:], bass.AP(spanf_d.tensor, 2 * N, [[0, P], [1, N]]))

        # ---- hidden tiles stream in on SP ----
        hid_sb = []
        for k in range(SB):
            t = hidp.tile([P, H], f32r, name=f"hid{k}", tag=f"hid{k}")
            nc.sync.dma_start(t[:], hid_d[ts(k, P), :])
            hid_sb.append(t)

        # ---- gpsimd: identity + iota, then gathers ----
        ident_f = consts.tile([P, P], f32)
        make_identity(nc, ident_f)
        ident = consts.tile([P, P], f32r)
        nc.vector.tensor_copy(ident[:], ident_f[:])
        iota_i = consts.tile([P, SB], i32)
        nc.gpsimd.iota(iota_i[:], pattern=[[P, SB]], base=0, channel_multiplier=1)
        iota_f = consts.tile([P, SB], f32)
        nc.vector.tensor_copy(iota_f[:], iota_i[:])

        # ---- remaining consts on the Activation ring ----
        bt1_sb = consts.tile([P, HB], f32)
        nc.scalar.dma_start(bt1_sb[:], bass.AP(bt1_d.tensor, 0, [[1, P], [P, HB]]))
        bs1_sb = consts.tile([P, HB], f32)
        nc.scalar.dma_start(bs1_sb[:], bass.AP(bs1_d.tensor, 0, [[1, P], [P, HB]]))
        bprop_sb = consts.tile([N_BIO, 1], f32)
        nc.scalar.dma_start(bprop_sb[:], bprop_d[:, :])
        bt2_sb = consts.tile([N_TYPE, 1], f32)
        nc.scalar.dma_start(bt2_sb[:], bt2_d[:, :])
        bs2_sb = consts.tile([N_SENS, 1], f32)
        nc.scalar.dma_start(bs2_sb[:], bs2_d[:, :])
        wprop_sb = consts.tile([P, HB, N_BIO], f32r)
        nc.scalar.dma_start(
            wprop_sb[:, :, :],
            bass.AP(wprop_d.tensor, 0, [[N_BIO, P], [N_BIO * P, HB], [1, N_BIO]]),
        )
        wt2_sb = consts.tile([P, HB, N_TYPE], f32r)
        nc.scalar.dma_start(
            wt2_sb[:, :, :],
            bass.AP(wt2_d.tensor, 0, [[N_TYPE, P], [N_TYPE * P, HB], [1, N_TYPE]]),
        )
        ws2_sb = consts.tile([P, HB, N_SENS], f32r)
        nc.scalar.dma_start(
            ws2_sb[:, :, :],
            bass.AP(ws2_d.tensor, 0, [[N_SENS, P], [N_SENS * P, HB], [1, N_SENS]]),
        )

        # ---- persistent rep^T tiles ----
        rt = [rtp.tile([P, N], f32r, name=f"rt{k}", tag=f"rt{k}") for k in range(KREP)]
        probsT = rtp.tile([N_TYPE, N], f32r, name="probsT", tag="probsT")

        # ---- persistent head-output tiles (tT and sT share slots) ----
        tT = [outs.tile([P, N], f32r, name=f"tT{m}", tag=f"hb{m}") for m in range(HB)]
        typeT_sb = outs.tile([N_TYPE, N], f32, name="typeT_sb", tag="typeT_sb")
        sensT_sb = outs.tile([N_SENS, N], f32, name="sensT_sb", tag="sensT_sb")
        probs = [
            outs.tile([P, N_TYPE], f32, name=f"probs{j}", tag=f"probs{j}")
            for j in range(NJ)
        ]

        # =========== Phase A: masks + pooled mask-matmul (starts as hidden streams in)
        with tc.tile_pool(name="maskp", bufs=1) as maskp, tc.tile_pool(
            name="psA", bufs=1, space="PSUM"
        ) as psA:
            gpsA = [
                psA.tile([P, N], f32, name=f"gpsA{m}", tag=f"gpsA{m}") for m in range(HB)
            ]
            m01s = []
            for k in range(SB):
                geS = maskp.tile([P, N], f32, name="geS", tag="geS", bufs=2)
                geE = maskp.tile([P, N], f32, name="geE", tag="geE", bufs=2)
                m01 = maskp.tile([P, N], f32r, name="m01", tag="m01", bufs=4)
                nc.vector.tensor_scalar(
                    out=geS[:], in0=startb[:], scalar1=iota_f[:, k : k + 1],
                    scalar2=None, op0=ALU.is_le,
                )
                nc.vector.tensor_scalar(
                    out=geE[:], in0=end1b[:], scalar1=iota_f[:, k : k + 1],
                    scalar2=None, op0=ALU.is_le,
                )
                nc.vector.tensor_tensor(out=m01[:], in0=geS[:], in1=geE[:], op=ALU.subtract)
                m01s.append(m01)
                for m in range(HB):
                    nc.tensor.matmul(
                        gpsA[m][:, :], hid_sb[k][:, ts(m, P)], m01[:],
                        start=(k == 0), stop=(k == SB - 1),
                    )
            for m in range(HB):
                nc.vector.tensor_tensor(
                    out=rt[2 * HB + m][:, :], in0=gpsA[m][:, :], in1=recipb[:], op=ALU.mult
                )

        # =========== Phase B: hidden^T + proposal, rep^T gathered parts
        with tc.tile_pool(name="gathp", bufs=1) as gathp, tc.tile_pool(
            name="htp", bufs=2
        ) as htp, tc.tile_pool(name="propsb", bufs=2) as propsbp, tc.tile_pool(
            name="psB", bufs=1, space="PSUM"
        ) as psB:
            sh, eh, wh = [], [], []
            for j in range(NJ):
                g = gathp.tile([P, H], f32r, name=f"sh{j}", tag="sh", bufs=3)
                nc.gpsimd.indirect_dma_start(
                    out=g[:], out_offset=None, in_=hid_d[:, :],
                    in_offset=bass.IndirectOffsetOnAxis(ap=idx_sb[:, 0, j : j + 1], axis=0),
                )
                sh.append(g)
                g = gathp.tile([P, H], f32r, name=f"eh{j}", tag="eh", bufs=3)
                nc.gpsimd.indirect_dma_start(
                    out=g[:], out_offset=None, in_=hid_d[:, :],
                    in_offset=bass.IndirectOffsetOnAxis(ap=idx_sb[:, 1, j : j + 1], axis=0),
                )
                eh.append(g)
                g = gathp.tile([P, HW], f32r, name=f"wh{j}", tag="wh", bufs=2)
                nc.gpsimd.indirect_dma_start(
                    out=g[:], out_offset=None, in_=wemb_d[:, :],
                    in_offset=bass.IndirectOffsetOnAxis(ap=idx_sb[:, 2, j : j + 1], axis=0),
                )
                wh.append(g)

            # hidden^T n-groups + proposal
            for n in range(NJ):
                hts = [
                    htp.tile([P, N], f32r, name=f"ht{h}", tag=f"ht{h}")
                    for h in range(HB)
                ]
                for kk in range(NJ):
                    k = n * NJ + kk
                    for h in range(HB):
                        ptr = psB.tile([P, P], f32r, name="ptr", tag="tr", bufs=6)
                        nc.tensor.transpose(ptr[:], hid_sb[k][:, ts(h, P)], ident[:])
                        nc.vector.tensor_copy(hts[h][:, ts(kk, P)], ptr[:])
                prop_ps = psB.tile([N_BIO, N], f32, name="prop_ps", tag="prop", bufs=2)
                for h in range(HB):
                    nc.tensor.matmul(
                        prop_ps[:, :], wprop_sb[:, h, :], hts[h][:, :],
                        start=(h == 0), stop=(h == HB - 1),
                    )
                psb = propsbp.tile([N_BIO, N], f32, name="psb", tag="psb")
                nc.scalar.activation(
                    psb[:, :], prop_ps[:, :], AF.Identity, bias=bprop_sb[:, 0:1], scale=1.0
                )
                nc.scalar.dma_start(
                    bass.AP(prop_o.tensor, n * N * N_BIO, [[1, N_BIO], [N_BIO, N]]),
                    psb[:, :],
                )

            # rep^T gathered parts: start (k 0-5), end (6-11), width (18-20)
            for j in range(NJ):
                for k in range(HB):
                    ptr = psB.tile([P, P], f32r, name="ptr", tag="tr", bufs=6)
                    nc.tensor.transpose(ptr[:], sh[j][:, ts(k, P)], ident[:])
                    nc.vector.tensor_copy(rt[k][:, ts(j, P)], ptr[:])
                for k in range(HB):
                    ptr = psB.tile([P, P], f32r, name="ptr", tag="tr", bufs=6)
                    nc.tensor.transpose(ptr[:], eh[j][:, ts(k, P)], ident[:])
                    nc.vector.tensor_copy(rt[HB + k][:, ts(j, P)], ptr[:])
                for k in range(HW // P):
                    ptr = psB.tile([P, P], f32r, name="ptr", tag="tr", bufs=6)
                    nc.tensor.transpose(ptr[:], wh[j][:, ts(k, P)], ident[:])
                    nc.vector.tensor_copy(rt[3 * HB + k][:, ts(j, P)], ptr[:])

        # =========== Phase C: GEMMs + heads
        with tc.tile_pool(name="softp", bufs=1) as softp, tc.tile_pool(
            name="wsp", bufs=1
        ) as wsp, tc.tile_pool(name="psmm", bufs=1, space="PSUM") as psmm, tc.tile_pool(
            name="pshead", bufs=1, space="PSUM"
        ) as pshead:
            gps = [
                psmm.tile([P, N], f32, name=f"gps{m}", tag=f"gps{m}") for m in range(HB)
            ]

            # GEMM1: t^T = gelu(W_t1^T @ rep^T + b_t1)
            # weights stream on the SP ring in 2-k-chunk super-tiles to
            # amortize the slot-WAR round-trip latency
            for kg in range((KREP + 1) // 2):
                nk = min(2, KREP - kg * 2)
                wt = wsp.tile([P, 2, H], f32r, name="wt1s", tag="wt1s", bufs=5)
                nc.sync.dma_start(
                    wt[:, :nk, :],
                    bass.AP(
                        wt1_d.tensor, kg * 2 * P * H, [[H, P], [P * H, nk], [1, H]]
                    ),
                )
                for kk in range(nk):
                    k = kg * 2 + kk
                    for m in range(HB):
                        nc.tensor.matmul(
                            gps[m][:, :], wt[:, kk, ts(m, P)], rt[k][:, :],
                            start=(k == 0), stop=(k == KREP - 1),
                        )
            for m in range(HB):
                nc.scalar.activation(
                    tT[m][:, :], gps[m][:, :], AF.Gelu, bias=bt1_sb[:, m : m + 1], scale=1.0
                )

            # type head
            typeT_ps = pshead.tile([N_TYPE, N], f32, name="typeT_ps", tag="headT")
            for k in range(HB):
                nc.tensor.matmul(
                    typeT_ps[:, :], wt2_sb[:, k, :], tT[k][:, :],
                    start=(k == 0), stop=(k == HB - 1),
                )
            nc.scalar.activation(
                typeT_sb[:, :], typeT_ps[:, :], AF.Identity, bias=bt2_sb[:, 0:1], scale=1.0
            )
            nc.scalar.dma_start(
                bass.AP(type_o.tensor, 0, [[1, N_TYPE], [N_TYPE, N]]), typeT_sb[:, :]
            )

            # softmax (span-major)
            for j in range(NJ):
                ttr = pshead.tile([P, N_TYPE], f32, name="ttr", tag="soft")
                nc.tensor.transpose(
                    ttr[:], typeT_sb[:, ts(j, P)], ident_f[:N_TYPE, :N_TYPE]
                )
                mx = softp.tile([P, 1], f32, name="mx", tag="mx", bufs=2)
                nc.vector.tensor_reduce(
                    out=mx[:], in_=ttr[:], axis=AX.X, op=ALU.max, negate=True
                )
                ex = softp.tile([P, N_TYPE], f32, name="ex", tag="ex", bufs=2)
                sm = softp.tile([P, 1], f32, name="sm", tag="sm", bufs=2)
                nc.scalar.activation(
                    ex[:], ttr[:], AF.Exp, bias=mx[:, 0:1], scale=1.0, accum_out=sm[:, 0:1]
                )
                rc = softp.tile([P, 1], f32, name="rc", tag="rc", bufs=2)
                nc.vector.reciprocal(rc[:], sm[:])
                nc.vector.tensor_scalar(
                    out=probs[j][:], in0=ex[:], scalar1=rc[:, 0:1], scalar2=None, op0=ALU.mult
                )

            # GEMM2 part A: rep^T rows (weights stream on SP ring after W_t1)
            for kg in range((KREP + 1) // 2):
                nk = min(2, KREP - kg * 2)
                ws = wsp.tile([P, 2, H], f32r, name="ws1s", tag="ws1s", bufs=5)
                nc.sync.dma_start(
                    ws[:, :nk, :],
                    bass.AP(
                        ws1_d.tensor, kg * 2 * P * H, [[H, P], [P * H, nk], [1, H]]
                    ),
                )
                for kk in range(nk):
                    k = kg * 2 + kk
                    for m in range(HB):
                        nc.tensor.matmul(
                            gps[m][:, :], ws[:, kk, ts(m, P)], rt[k][:, :],
                            start=(k == 0), stop=False,
                        )

            # probs^T chunk + final GEMM2 accumulation
            for j in range(NJ):
                ptr2 = pshead.tile([N_TYPE, P], f32, name="ptr2", tag="soft")
                nc.tensor.transpose(ptr2[:], probs[j][:, :], ident_f[:, :])
                nc.vector.tensor_copy(probsT[:, ts(j, P)], ptr2[:])
            ws_last = wsp.tile([P, H], f32r, name="wslast", tag="wslast", bufs=1)
            nc.sync.dma_start(ws_last[:N_TYPE, :], ws1_d[SPAN_DIM : SPAN_DIM + N_TYPE, :])
            for m in range(HB):
                nc.tensor.matmul(
                    gps[m][:, :], ws_last[:N_TYPE, ts(m, P)], probsT[:, :],
                    start=False, stop=True,
                )
            sT = [outs.tile([P, N], f32r, name=f"sT{m}", tag=f"hb{m}") for m in range(HB)]
            for m in range(HB):
                nc.scalar.activation(
                    sT[m][:, :], gps[m][:, :], AF.Gelu, bias=bs1_sb[:, m : m + 1], scale=1.0
                )

            # sensitivity head
            sensT_ps = pshead.tile([N_SENS, N], f32, name="sensT_ps", tag="headT")
            for k in range(HB):
                nc.tensor.matmul(
                    sensT_ps[:, :], ws2_sb[:, k, :], sT[k][:, :],
                    start=(k == 0), stop=(k == HB - 1),
                )
            nc.scalar.activation(
                sensT_sb[:, :], sensT_ps[:, :], AF.Identity, bias=bs2_sb[:, 0:1], scale=1.0
            )
            nc.scalar.dma_start(
                bass.AP(sens_o.tensor, 0, [[1, N_SENS], [N_SENS, N]]), sensT_sb[:, :]
            )

    nc.compile()
    return nc


_NC_CACHE = None


def _get_program():
    global _NC_CACHE
    if _NC_CACHE is None:
        _NC_CACHE = _build_program()
    return _NC_CACHE


def _make_in_maps(inputs):
    hidden = np.asarray(inputs["hidden"], dtype=np.float32)
    spans = np.asarray(inputs["candidate_spans"], dtype=np.int64)
    width_emb = np.ascontiguousarray(np.asarray(inputs["width_emb"], np.float32))
    W_prop = np.ascontiguousarray(np.asarray(inputs["W_prop"], np.float32))
    b_prop = np.asarray(inputs["b_prop"], np.float32).reshape(N_BIO, 1)
    W_t1 = np.ascontiguousarray(np.asarray(inputs["W_t1"], np.float32))
    b_t1 = np.asarray(inputs["b_t1"], np.float32).reshape(HB, P)
    W_t2 = np.ascontiguousarray(np.asarray(inputs["W_t2"], np.float32))
    b_t2 = np.asarray(inputs["b_t2"], np.float32).reshape(N_TYPE, 1)
    W_s1 = np.ascontiguousarray(np.asarray(inputs["W_s1"], np.float32))
    b_s1 = np.asarray(inputs["b_s1"], np.float32).reshape(HB, P)
    W_s2 = np.ascontiguousarray(np.asarray(inputs["W_s2"], np.float32))
    b_s2 = np.asarray(inputs["b_s2"], np.float32).reshape(N_SENS, 1)

    starts = spans[..., 0]
    ends = spans[..., 1]
    widths = ends - starts + 1
    in_maps = []
    for b in range(B):
        gidx = np.stack(
            [starts[b], ends[b], np.minimum(widths[b], WIDTH_VOCAB - 1)]
        ).astype(np.int32)
        spanf = np.stack(
            [
                starts[b].astype(np.float32),
                (ends[b] + 1).astype(np.float32),
                (1.0 / widths[b]).astype(np.float32),
            ]
        ).astype(np.float32)
        in_maps.append(
            {
                "hidden": np.ascontiguousarray(hidden[b]),
                "gidx": gidx,
                "spanf": spanf,
                "width_emb": width_emb,
                "W_prop": W_prop,
                "b_prop": b_prop,
                "W_t1": W_t1,
                "b_t1": b_t1,
                "W_t2": W_t2,
                "b_t2": b_t2,
                "W_s1": W_s1,
                "b_s1": b_s1,
                "W_s2": W_s2,
                "b_s2": b_s2,
            }
        )
    return in_maps


def run(inputs, trace=False):
    nc = _get_program()
    in_maps = _make_in_maps(inputs)
    res = run_bass_kernel_spmd(nc, in_maps, core_ids=list(range(B)), trace=trace)
    prop = np.stack([res.results[b]["prop_out"] for b in range(B)])
    typ = np.stack([res.results[b]["type_out"] for b in range(B)])
    sens = np.stack([res.results[b]["sens_out"] for b in range(B)])
    return (prop, typ, sens), res


def kernel(**inputs):
    out, _ = run(inputs, trace=False)
    return out


# revision 8
# speedup vs baseline: 1.8794x; 1.3929x over previous
"""Trainium2 Bass kernel for nn_MultiHeadPiiModel (segment_reduce).

Data-parallel over batch: 8 NeuronCores, one batch element each.
Per-core pipeline (all matmuls in fp32r):
  - proposal head: hidden^T via PE transposes, prop^T = W_prop^T @ hidden^T
  - span gathers (start/end/width rows) via indirect DMA
  - ragged mean-pool as a mask matmul: pooled^T = hidden^T @ mask01 * (1/w)
  - rep^T assembled feature-major; two big GEMMs with W stationary
  - type head + softmax + sensitivity head, outputs written transposed

DMA queue split: hidden + W_t1 stream on the SP (sync) HWDGE ring,
consts + W_s1 stream + outputs on the Activation ring, gathers on SWDGE.
"""

import sys
import contextlib

sys.path.insert(0, "/opt/trn_rl_repo")

import numpy as np
import concourse.bass as bass
import concourse.mybir as mybir
import concourse.tile as tile
from concourse import bacc
from concourse.bass import ts
from concourse.bass_utils import run_bass_kernel_spmd
from concourse.masks import make_identity

f32 = mybir.dt.float32
f32r = mybir.dt.float32r
i32 = mybir.dt.int32
AF = mybir.ActivationFunctionType
ALU = mybir.AluOpType
AX = mybir.AxisListType

B, S, H = 8, 2048, 768
N = 512  # spans
HW = 384  # width emb dim = H // 2
SPAN_DIM = H * 3 + HW  # 2688
N_BIO, N_TYPE, N_SENS = 3, 18, 4
P = 128
SB = S // P  # 16 s-blocks
HB = H // P  # 6 h-chunks
NJ = N // P  # 4 span-chunks
KREP = SPAN_DIM // P  # 21 rep k-chunks
WIDTH_VOCAB = 64


def _build_program():
    nc = bacc.Bacc("TRN2", target_bir_lowering=False, debug=False, num_devices=B)

    hid_d = nc.dram_tensor("hidden", [S, H], f32r, kind="ExternalInput").ap()
    gidx_d = nc.dram_tensor("gidx", [3, N], i32, kind="ExternalInput").ap()
    spanf_d = nc.dram_tensor("spanf", [3, N], f32, kind="ExternalInput").ap()
    wemb_d = nc.dram_tensor("width_emb", [WIDTH_VOCAB, HW], f32r, kind="ExternalInput").ap()
    wprop_d = nc.dram_tensor("W_prop", [H, N_BIO], f32r, kind="ExternalInput").ap()
    bprop_d = nc.dram_tensor("b_prop", [N_BIO, 1], f32, kind="ExternalInput").ap()
    wt1_d = nc.dram_tensor("W_t1", [SPAN_DIM, H], f32r, kind="ExternalInput").ap()
    bt1_d = nc.dram_tensor("b_t1", [HB, P], f32, kind="ExternalInput").ap()
    wt2_d = nc.dram_tensor("W_t2", [H, N_TYPE], f32r, kind="ExternalInput").ap()
    bt2_d = nc.dram_tensor("b_t2", [N_TYPE, 1], f32, kind="ExternalInput").ap()
    ws1_d = nc.dram_tensor("W_s1", [SPAN_DIM + N_TYPE, H], f32r, kind="ExternalInput").ap()
    bs1_d = nc.dram_tensor("b_s1", [HB, P], f32, kind="ExternalInput").ap()
    ws2_d = nc.dram_tensor("W_s2", [H, N_SENS], f32r, kind="ExternalInput").ap()
    bs2_d = nc.dram_tensor("b_s2", [N_SENS, 1], f32, kind="ExternalInput").ap()

    prop_o = nc.dram_tensor("prop_out", [N_BIO, S], f32, kind="ExternalOutput").ap()
    type_o = nc.dram_tensor("type_out", [N_TYPE, N], f32, kind="ExternalOutput").ap()
    sens_o = nc.dram_tensor("sens_out", [N_SENS, N], f32, kind="ExternalOutput").ap()

    with tile.TileContext(nc) as tc, contextlib.ExitStack() as ctx:
        consts = ctx.enter_context(tc.tile_pool(name="consts", bufs=1))
        hidp = ctx.enter_context(tc.tile_pool(name="hidp", bufs=1))
        rtp = ctx.enter_context(tc.tile_pool(name="rtp", bufs=1))
        outs = ctx.enter_context(tc.tile_pool(name="outs", bufs=1))

        # ---- early small inputs on the SP ring (before hidden) ----
        idx_sb = consts.tile([P, 3, NJ], i32)
        nc.sync.dma_start(
            idx_sb[:, :, :], bass.AP(gidx_d.tensor, 0, [[1, P], [N, 3], [P, NJ]])
        )
        startb = consts.tile([P, N], f32)
        end1b = consts.tile([P, N], f32)
        recipb = consts.tile([P, N], f32)
        nc.sync.dma_start(startb[:], bass.AP(spanf_d.tensor, 0 * N, [[0, P], [1, N]]))
        nc.sync.dma_start(end1b[:], bass.AP(spanf_d.tensor, 1 * N, [[0, P], [1, N]]))
        nc.sync.dma_start(recipb[:], bass.AP(spanf_d.tensor, 2 * N, [[0, P], [1, N]]))

        # ---- hidden tiles stream in on SP ----
        hid_sb = []
        for k in range(SB):
            t = hidp.tile([P, H], f32r, name=f"hid{k}", tag=f"hid{k}")
            nc.sync.dma_start(t[:], hid_d[ts(k, P), :])
            hid_sb.append(t)

        # ---- gpsimd: identity + iota, then gathers ----
        ident_f = consts.tile([P, P], f32)
        make_identity(nc, ident_f)
        ident = consts.tile([P, P], f32r)
        nc.vector.tensor_copy(ident[:], ident_f[:])
        iota_i = consts.tile([P, SB], i32)
        nc.gpsimd.iota(iota_i[:], pattern=[[P, SB]], base=0, channel_multiplier=1)
        iota_f = consts.tile([P, SB], f32)
        nc.vector.tensor_copy(iota_f[:], iota_i[:])

        # ---- remaining consts on the Activation ring ----
        bt1_sb = consts.tile([P, HB], f32)
        nc.scalar.dma_start(bt1_sb[:], bass.AP(bt1_d.tensor, 0, [[1, P], [P, HB]]))
        bs1_sb = consts.tile([P, HB], f32)
        nc.scalar.dma_start(bs1_sb[:], bass.AP(bs1_d.tensor, 0, [[1, P], [P, HB]]))
        bprop_sb = consts.tile([N_BIO, 1], f32)
        nc.scalar.dma_start(bprop_sb[:], bprop_d[:, :])
        bt2_sb = consts.tile([N_TYPE, 1], f32)
        nc.scalar.dma_start(bt2_sb[:], bt2_d[:, :])
        bs2_sb = consts.tile([N_SENS, 1], f32)
        nc.scalar.dma_start(bs2_sb[:], bs2_d[:, :])
        wprop_sb = consts.tile([P, HB, N_BIO], f32r)
        nc.scalar.dma_start(
            wprop_sb[:, :, :],
            bass.AP(wprop_d.tensor, 0, [[N_BIO, P], [N_BIO * P, HB], [1, N_BIO]]),
        )
        wt2_sb = consts.tile([P, HB, N_TYPE], f32r)
        nc.scalar.dma_start(
            wt2_sb[:, :, :],
            bass.AP(wt2_d.tensor, 0, [[N_TYPE, P], [N_TYPE * P, HB], [1, N_TYPE]]),
        )
        ws2_sb = consts.tile([P, HB, N_SENS], f32r)
        nc.scalar.dma_start(
            ws2_sb[:, :, :],
            bass.AP(ws2_d.tensor, 0, [[N_SENS, P], [N_SENS * P, HB], [1, N_SENS]]),
        )

        # ---- persistent rep^T tiles ----
        rt = [rtp.tile([P, N], f32r, name=f"rt{k}", tag=f"rt{k}") for k in range(KREP)]
        probsT = rtp.tile([N_TYPE, N], f32r, name="probsT", tag="probsT")

        # ---- persistent head-output tiles (tT and sT share slots) ----
        tT = [outs.tile([P, N], f32r, name=f"tT{m}", tag=f"hb{m}") for m in range(HB)]
        typeT_sb = outs.tile([N_TYPE, N], f32, name="typeT_sb", tag="typeT_sb")
        sensT_sb = outs.tile([N_SENS, N], f32, name="sensT_sb", tag="sensT_sb")
        probs = [
            outs.tile([P, N_TYPE], f32, name=f"probs{j}", tag=f"probs{j}")
            for j in range(NJ)
        ]

        # =========== Phase A: masks + pooled mask-matmul (starts as hidden streams in)
        with tc.tile_pool(name="maskp", bufs=1) as maskp, tc.tile_pool(
            name="psA", bufs=1, space="PSUM"
        ) as psA:
            gpsA = [
                psA.tile([P, N], f32, name=f"gpsA{m}", tag=f"gpsA{m}") for m in range(HB)
            ]
            m01s = []
            for k in range(SB):
                geS = maskp.tile([P, N], f32, name="geS", tag="geS", bufs=2)
                geE = maskp.tile([P, N], f32, name="geE", tag="geE", bufs=2)
                m01 = maskp.tile([P, N], f32r, name="m01", tag="m01", bufs=4)
                nc.vector.tensor_scalar(
                    out=geS[:], in0=startb[:], scalar1=iota_f[:, k : k + 1],
                    scalar2=None, op0=ALU.is_le,
                )
                nc.vector.tensor_scalar(
                    out=geE[:], in0=end1b[:], scalar1=iota_f[:, k : k + 1],
                    scalar2=None, op0=ALU.is_le,
                )
                nc.vector.tensor_tensor(out=m01[:], in0=geS[:], in1=geE[:], op=ALU.subtract)
                m01s.append(m01)
                for m in range(HB):
                    nc.tensor.matmul(
                        gpsA[m][:, :], hid_sb[k][:, ts(m, P)], m01[:],
                        start=(k == 0), stop=(k == SB - 1),
                    )
            for m in range(HB):
                nc.vector.tensor_tensor(
                    out=rt[2 * HB + m][:, :], in0=gpsA[m][:, :], in1=recipb[:], op=ALU.mult
                )

        # =========== Phase B: hidden^T + proposal, rep^T gathered parts
        with tc.tile_pool(name="gathp", bufs=1) as gathp, tc.tile_pool(
            name="htp", bufs=2
        ) as htp, tc.tile_pool(name="propsb", bufs=2) as propsbp, tc.tile_pool(
            name="psB", bufs=1, space="PSUM"
        ) as psB:
            sh, eh, wh = [], [], []
            for j in range(NJ):
                g = gathp.tile([P, H], f32r, name=f"sh{j}", tag="sh", bufs=4)
                nc.gpsimd.indirect_dma_start(
                    out=g[:], out_offset=None, in_=hid_d[:, :],
                    in_offset=bass.IndirectOffsetOnAxis(ap=idx_sb[:, 0, j : j + 1], axis=0),
                )
                sh.append(g)
                g = gathp.tile([P, H], f32r, name=f"eh{j}", tag="eh", bufs=4)
                nc.gpsimd.indirect_dma_start(
                    out=g[:], out_offset=None, in_=hid_d[:, :],
                    in_offset=bass.IndirectOffsetOnAxis(ap=idx_sb[:, 1, j : j + 1], axis=0),
                )
                eh.append(g)
                g = gathp.tile([P, HW], f32r, name=f"wh{j}", tag="wh", bufs=4)
                nc.gpsimd.indirect_dma_start(
                    out=g[:], out_offset=None, in_=wemb_d[:, :],
                    in_offset=bass.IndirectOffsetOnAxis(ap=idx_sb[:, 2, j : j + 1], axis=0),
                )
                wh.append(g)

            # hidden^T n-groups + proposal
            for n in range(NJ):
                hts = [
                    htp.tile([P, N], f32r, name=f"ht{h}", tag=f"ht{h}")
                    for h in range(HB)
                ]
                for kk in range(NJ):
                    k = n * NJ + kk
                    for h in range(HB):
                        ptr = psB.tile([P, P], f32r, name="ptr", tag="tr", bufs=6)
                        nc.tensor.transpose(ptr[:], hid_sb[k][:, ts(h, P)], ident[:])
                        nc.vector.tensor_copy(hts[h][:, ts(kk, P)], ptr[:])
                prop_ps = psB.tile([N_BIO, N], f32, name="prop_ps", tag="prop", bufs=2)
                for h in range(HB):
                    nc.tensor.matmul(
                        prop_ps[:, :], wprop_sb[:, h, :], hts[h][:, :],
                        start=(h == 0), stop=(h == HB - 1),
                    )
                psb = propsbp.tile([N_BIO, N], f32, name="psb", tag="psb")
                nc.scalar.activation(
                    psb[:, :], prop_ps[:, :], AF.Identity, bias=bprop_sb[:, 0:1], scale=1.0
                )
                nc.scalar.dma_start(prop_o[:, ts(n, N)], psb[:, :])

            # rep^T gathered parts: start (k 0-5), end (6-11), width (18-20)
            for j in range(NJ):
                for k in range(HB):
                    ptr = psB.tile([P, P], f32r, name="ptr", tag="tr", bufs=6)
                    nc.tensor.transpose(ptr[:], sh[j][:, ts(k, P)], ident[:])
                    nc.vector.tensor_copy(rt[k][:, ts(j, P)], ptr[:])
                for k in range(HB):
                    ptr = psB.tile([P, P], f32r, name="ptr", tag="tr", bufs=6)
                    nc.tensor.transpose(ptr[:], eh[j][:, ts(k, P)], ident[:])
                    nc.vector.tensor_copy(rt[HB + k][:, ts(j, P)], ptr[:])
                for k in range(HW // P):
                    ptr = psB.tile([P, P], f32r, name="ptr", tag="tr", bufs=6)
                    nc.tensor.transpose(ptr[:], wh[j][:, ts(k, P)], ident[:])
                    nc.vector.tensor_copy(rt[3 * HB + k][:, ts(j, P)], ptr[:])

        # =========== Phase C: GEMMs + heads
        with tc.tile_pool(name="softp", bufs=1) as softp, tc.tile_pool(
            name="wsp", bufs=1
        ) as wsp, tc.tile_pool(name="psmm", bufs=1, space="PSUM") as psmm, tc.tile_pool(
            name="pshead", bufs=1, space="PSUM"
        ) as pshead:
            gps = [
                psmm.tile([P, N], f32, name=f"gps{m}", tag=f"gps{m}") for m in range(HB)
            ]

            # GEMM1: t^T = gelu(W_t1^T @ rep^T + b_t1)
            # weights stream on the SP ring in 2-k-chunk super-tiles to
            # amortize the slot-WAR round-trip latency
            for kg in range((KREP + 1) // 2):
                nk = min(2, KREP - kg * 2)
                wt = wsp.tile([P, 2, H], f32r, name="wt1s", tag="wt1s", bufs=5)
                nc.sync.dma_start(
                    wt[:, :nk, :],
                    bass.AP(
                        wt1_d.tensor, kg * 2 * P * H, [[H, P], [P * H, nk], [1, H]]
                    ),
                )
                for kk in range(nk):
                    k = kg * 2 + kk
                    for m in range(HB):
                        nc.tensor.matmul(
                            gps[m][:, :], wt[:, kk, ts(m, P)], rt[k][:, :],
                            start=(k == 0), stop=(k == KREP - 1),
                        )
            for m in range(HB):
                nc.scalar.activation(
                    tT[m][:, :], gps[m][:, :], AF.Gelu, bias=bt1_sb[:, m : m + 1], scale=1.0
                )

            # type head
            typeT_ps = pshead.tile([N_TYPE, N], f32, name="typeT_ps", tag="headT")
            for k in range(HB):
                nc.tensor.matmul(
                    typeT_ps[:, :], wt2_sb[:, k, :], tT[k][:, :],
                    start=(k == 0), stop=(k == HB - 1),
                )
            nc.scalar.activation(
                typeT_sb[:, :], typeT_ps[:, :], AF.Identity, bias=bt2_sb[:, 0:1], scale=1.0
            )
            nc.scalar.dma_start(type_o[:, :], typeT_sb[:, :])

            # softmax (span-major)
            for j in range(NJ):
                ttr = pshead.tile([P, N_TYPE], f32, name="ttr", tag="soft")
                nc.tensor.transpose(
                    ttr[:], typeT_sb[:, ts(j, P)], ident_f[:N_TYPE, :N_TYPE]
                )
                mx = softp.tile([P, 1], f32, name="mx", tag="mx", bufs=2)
                nc.vector.tensor_reduce(
                    out=mx[:], in_=ttr[:], axis=AX.X, op=ALU.max, negate=True
                )
                ex = softp.tile([P, N_TYPE], f32, name="ex", tag="ex", bufs=2)
                sm = softp.tile([P, 1], f32, name="sm", tag="sm", bufs=2)
                nc.scalar.activation(
                    ex[:], ttr[:], AF.Exp, bias=mx[:, 0:1], scale=1.0, accum_out=sm[:, 0:1]
                )
                rc = softp.tile([P, 1], f32, name="rc", tag="rc", bufs=2)
                nc.vector.reciprocal(rc[:], sm[:])
                nc.vector.tensor_scalar(
                    out=probs[j][:], in0=ex[:], scalar1=rc[:, 0:1], scalar2=None, op0=ALU.mult
                )

            # GEMM2 part A: rep^T rows (weights stream on SP ring after W_t1)
            for kg in range((KREP + 1) // 2):
                nk = min(2, KREP - kg * 2)
                ws = wsp.tile([P, 2, H], f32r, name="ws1s", tag="ws1s", bufs=5)
                nc.sync.dma_start(
                    ws[:, :nk, :],
                    bass.AP(
                        ws1_d.tensor, kg * 2 * P * H, [[H, P], [P * H, nk], [1, H]]
                    ),
                )
                for kk in range(nk):
                    k = kg * 2 + kk
                    for m in range(HB):
                        nc.tensor.matmul(
                            gps[m][:, :], ws[:, kk, ts(m, P)], rt[k][:, :],
                            start=(k == 0), stop=False,
                        )

            # probs^T chunk + final GEMM2 accumulation
            for j in range(NJ):
                ptr2 = pshead.tile([N_TYPE, P], f32, name="ptr2", tag="soft")
                nc.tensor.transpose(ptr2[:], probs[j][:, :], ident_f[:, :])
                nc.vector.tensor_copy(probsT[:, ts(j, P)], ptr2[:])
            ws_last = wsp.tile([P, H], f32r, name="wslast", tag="wslast", bufs=1)
            nc.sync.dma_start(ws_last[:N_TYPE, :], ws1_d[SPAN_DIM : SPAN_DIM + N_TYPE, :])
            for m in range(HB):
                nc.tensor.matmul(
                    gps[m][:, :], ws_last[:N_TYPE, ts(m, P)], probsT[:, :],
                    start=False, stop=True,
                )
            sT = [outs.tile([P, N], f32r, name=f"sT{m}", tag=f"hb{m}") for m in range(HB)]
            for m in range(HB):
                nc.scalar.activation(
                    sT[m][:, :], gps[m][:, :], AF.Gelu, bias=bs1_sb[:, m : m + 1], scale=1.0
                )

            # sensitivity head
            sensT_ps = pshead.tile([N_SENS, N], f32, name="sensT_ps", tag="headT")
            for k in range(HB):
                nc.tensor.matmul(
                    sensT_ps[:, :], ws2_sb[:, k, :], sT[k][:, :],
                    start=(k == 0), stop=(k == HB - 1),
                )
            nc.scalar.activation(
                sensT_sb[:, :], sensT_ps[:, :], AF.Identity, bias=bs2_sb[:, 0:1], scale=1.0
            )
            nc.scalar.dma_start(sens_o[:, :], sensT_sb[:, :])

    nc.compile()
    return nc


_NC_CACHE = None


def _get_program():
    global _NC_CACHE
    if _NC_CACHE is None:
        _NC_CACHE = _build_program()
    return _NC_CACHE


def _make_in_maps(inputs):
    hidden = np.asarray(inputs["hidden"], dtype=np.float32)
    spans = np.asarray(inputs["candidate_spans"], dtype=np.int64)
    width_emb = np.ascontiguousarray(np.asarray(inputs["width_emb"], np.float32))
    W_prop = np.ascontiguousarray(np.asarray(inputs["W_prop"], np.float32))
    b_prop = np.asarray(inputs["b_prop"], np.float32).reshape(N_BIO, 1)
    W_t1 = np.ascontiguousarray(np.asarray(inputs["W_t1"], np.float32))
    b_t1 = np.asarray(inputs["b_t1"], np.float32).reshape(HB, P)
    W_t2 = np.ascontiguousarray(np.asarray(inputs["W_t2"], np.float32))
    b_t2 = np.asarray(inputs["b_t2"], np.float32).reshape(N_TYPE, 1)
    W_s1 = np.ascontiguousarray(np.asarray(inputs["W_s1"], np.float32))
    b_s1 = np.asarray(inputs["b_s1"], np.float32).reshape(HB, P)
    W_s2 = np.ascontiguousarray(np.asarray(inputs["W_s2"], np.float32))
    b_s2 = np.asarray(inputs["b_s2"], np.float32).reshape(N_SENS, 1)

    starts = spans[..., 0]
    ends = spans[..., 1]
    widths = ends - starts + 1
    in_maps = []
    for b in range(B):
        gidx = np.stack(
            [starts[b], ends[b], np.minimum(widths[b], WIDTH_VOCAB - 1)]
        ).astype(np.int32)
        spanf = np.stack(
            [
                starts[b].astype(np.float32),
                (ends[b] + 1).astype(np.float32),
                (1.0 / widths[b]).astype(np.float32),
            ]
        ).astype(np.float32)
        in_maps.append(
            {
                "hidden": np.ascontiguousarray(hidden[b]),
                "gidx": gidx,
                "spanf": spanf,
                "width_emb": width_emb,
                "W_prop": W_prop,
                "b_prop": b_prop,
                "W_t1": W_t1,
                "b_t1": b_t1,
                "W_t2": W_t2,
                "b_t2": b_t2,
                "W_s1": W_s1,
                "b_s1": b_s1,
                "W_s2": W_s2,
                "b_s2": b_s2,
            }
        )
    return in_maps


def run(inputs, trace=False):
    nc = _get_program()
    in_maps = _make_in_maps(inputs)
    res = run_bass_kernel_spmd(nc, in_maps, core_ids=list(range(B)), trace=trace)
    prop = np.stack([res.results[b]["prop_out"].T for b in range(B)])
    typ = np.stack([res.results[b]["type_out"].T for b in range(B)])
    sens = np.stack([res.results[b]["sens_out"].T for b in range(B)])
    return (prop, typ, sens), res


def kernel(**inputs):
    out, _ = run(inputs, trace=False)
    return out


# revision 9
# speedup vs baseline: 1.8974x; 1.0096x over previous
"""Trainium2 Bass kernel for nn_MultiHeadPiiModel (segment_reduce).

Data-parallel over batch: 8 NeuronCores, one batch element each.
Per-core pipeline (all matmuls in fp32r):
  - proposal head: hidden^T via PE transposes, prop^T = W_prop^T @ hidden^T
  - span gathers (start/end/width rows) via indirect DMA
  - ragged mean-pool as a mask matmul: pooled^T = hidden^T @ mask01 * (1/w)
  - rep^T assembled feature-major; two big GEMMs with W stationary
  - type head + softmax + sensitivity head, outputs written transposed

DMA queue split: hidden + W_t1 stream on the SP (sync) HWDGE ring,
consts + W_s1 stream + outputs on the Activation ring, gathers on SWDGE.
"""

import sys
import contextlib

sys.path.insert(0, "/opt/trn_rl_repo")

import numpy as np
import concourse.bass as bass
import concourse.mybir as mybir
import concourse.tile as tile
from concourse import bacc
from concourse.bass import ts
from concourse.bass_utils import run_bass_kernel_spmd
from concourse.masks import make_identity

f32 = mybir.dt.float32
f32r = mybir.dt.float32r
i32 = mybir.dt.int32
AF = mybir.ActivationFunctionType
ALU = mybir.AluOpType
AX = mybir.AxisListType

B, S, H = 8, 2048, 768
N = 512  # spans
HW = 384  # width emb dim = H // 2
SPAN_DIM = H * 3 + HW  # 2688
N_BIO, N_TYPE, N_SENS = 3, 18, 4
P = 128
SB = S // P  # 16 s-blocks
HB = H // P  # 6 h-chunks
NJ = N // P  # 4 span-chunks
KREP = SPAN_DIM // P  # 21 rep k-chunks
WIDTH_VOCAB = 64


def _build_program():
    nc = bacc.Bacc("TRN2", target_bir_lowering=False, debug=False, num_devices=B)

    hid_d = nc.dram_tensor("hidden", [S, H], f32r, kind="ExternalInput").ap()
    gidx_d = nc.dram_tensor("gidx", [3, N], i32, kind="ExternalInput").ap()
    spanf_d = nc.dram_tensor("spanf", [3, N], f32, kind="ExternalInput").ap()
    wemb_d = nc.dram_tensor("width_emb", [WIDTH_VOCAB, HW], f32r, kind="ExternalInput").ap()
    wprop_d = nc.dram_tensor("W_prop", [H, N_BIO], f32r, kind="ExternalInput").ap()
    bprop_d = nc.dram_tensor("b_prop", [N_BIO, 1], f32, kind="ExternalInput").ap()
    wt1_d = nc.dram_tensor("W_t1", [SPAN_DIM, H], f32r, kind="ExternalInput").ap()
    bt1_d = nc.dram_tensor("b_t1", [HB, P], f32, kind="ExternalInput").ap()
    wt2_d = nc.dram_tensor("W_t2", [H, N_TYPE], f32r, kind="ExternalInput").ap()
    bt2_d = nc.dram_tensor("b_t2", [N_TYPE, 1], f32, kind="ExternalInput").ap()
    ws1_d = nc.dram_tensor("W_s1", [SPAN_DIM + N_TYPE, H], f32r, kind="ExternalInput").ap()
    bs1_d = nc.dram_tensor("b_s1", [HB, P], f32, kind="ExternalInput").ap()
    ws2_d = nc.dram_tensor("W_s2", [H, N_SENS], f32r, kind="ExternalInput").ap()
    bs2_d = nc.dram_tensor("b_s2", [N_SENS, 1], f32, kind="ExternalInput").ap()

    prop_o = nc.dram_tensor("prop_out", [N_BIO, S], f32, kind="ExternalOutput").ap()
    type_o = nc.dram_tensor("type_out", [N_TYPE, N], f32, kind="ExternalOutput").ap()
    sens_o = nc.dram_tensor("sens_out", [N_SENS, N], f32, kind="ExternalOutput").ap()

    with tile.TileContext(nc) as tc, contextlib.ExitStack() as ctx:
        consts = ctx.enter_context(tc.tile_pool(name="consts", bufs=1))
        hidp = ctx.enter_context(tc.tile_pool(name="hidp", bufs=1))
        rtp = ctx.enter_context(tc.tile_pool(name="rtp", bufs=1))
        outs = ctx.enter_context(tc.tile_pool(name="outs", bufs=1))

        # ---- early small inputs on the SP ring (before hidden) ----
        startb = consts.tile([P, N], f32)
        end1b = consts.tile([P, N], f32)
        recipb = consts.tile([P, N], f32)
        nc.sync.dma_start(startb[:], bass.AP(spanf_d.tensor, 0 * N, [[0, P], [1, N]]))
        nc.sync.dma_start(end1b[:], bass.AP(spanf_d.tensor, 1 * N, [[0, P], [1, N]]))
        nc.sync.dma_start(recipb[:], bass.AP(spanf_d.tensor, 2 * N, [[0, P], [1, N]]))
        idx_sb = consts.tile([P, 3, NJ], i32)
        nc.sync.dma_start(
            idx_sb[:, :, :], bass.AP(gidx_d.tensor, 0, [[1, P], [N, 3], [P, NJ]])
        )

        # ---- hidden tiles stream in on SP ----
        hid_sb = []
        for k in range(SB):
            t = hidp.tile([P, H], f32r, name=f"hid{k}", tag=f"hid{k}")
            nc.sync.dma_start(t[:], hid_d[ts(k, P), :])
            hid_sb.append(t)

        # ---- gpsimd: identity + iota, then gathers ----
        ident_f = consts.tile([P, P], f32)
        make_identity(nc, ident_f)
        ident = consts.tile([P, P], f32r)
        nc.vector.tensor_copy(ident[:], ident_f[:])
        iota_i = consts.tile([P, SB], i32)
        nc.gpsimd.iota(iota_i[:], pattern=[[P, SB]], base=0, channel_multiplier=1)
        iota_f = consts.tile([P, SB], f32)
        nc.vector.tensor_copy(iota_f[:], iota_i[:])

        # ---- remaining consts on the Activation ring ----
        bt1_sb = consts.tile([P, HB], f32)
        nc.scalar.dma_start(bt1_sb[:], bass.AP(bt1_d.tensor, 0, [[1, P], [P, HB]]))
        bs1_sb = consts.tile([P, HB], f32)
        nc.scalar.dma_start(bs1_sb[:], bass.AP(bs1_d.tensor, 0, [[1, P], [P, HB]]))
        bprop_sb = consts.tile([N_BIO, 1], f32)
        nc.scalar.dma_start(bprop_sb[:], bprop_d[:, :])
        bt2_sb = consts.tile([N_TYPE, 1], f32)
        nc.scalar.dma_start(bt2_sb[:], bt2_d[:, :])
        bs2_sb = consts.tile([N_SENS, 1], f32)
        nc.scalar.dma_start(bs2_sb[:], bs2_d[:, :])
        wprop_sb = consts.tile([P, HB, N_BIO], f32r)
        nc.scalar.dma_start(
            wprop_sb[:, :, :],
            bass.AP(wprop_d.tensor, 0, [[N_BIO, P], [N_BIO * P, HB], [1, N_BIO]]),
        )
        wt2_sb = consts.tile([P, HB, N_TYPE], f32r)
        nc.scalar.dma_start(
            wt2_sb[:, :, :],
            bass.AP(wt2_d.tensor, 0, [[N_TYPE, P], [N_TYPE * P, HB], [1, N_TYPE]]),
        )
        ws2_sb = consts.tile([P, HB, N_SENS], f32r)
        nc.scalar.dma_start(
            ws2_sb[:, :, :],
            bass.AP(ws2_d.tensor, 0, [[N_SENS, P], [N_SENS * P, HB], [1, N_SENS]]),
        )

        # ---- persistent rep^T tiles ----
        rt = [rtp.tile([P, N], f32r, name=f"rt{k}", tag=f"rt{k}") for k in range(KREP)]
        probsT = rtp.tile([N_TYPE, N], f32r, name="probsT", tag="probsT")

        # ---- persistent head-output tiles (tT and sT share slots) ----
        tT = [outs.tile([P, N], f32r, name=f"tT{m}", tag=f"hb{m}") for m in range(HB)]
        typeT_sb = outs.tile([N_TYPE, N], f32, name="typeT_sb", tag="typeT_sb")
        sensT_sb = outs.tile([N_SENS, N], f32, name="sensT_sb", tag="sensT_sb")
        probs = [
            outs.tile([P, N_TYPE], f32, name=f"probs{j}", tag=f"probs{j}")
            for j in range(NJ)
        ]

        # =========== Phase A: masks + pooled mask-matmul (starts as hidden streams in)
        with tc.tile_pool(name="maskp", bufs=1) as maskp, tc.tile_pool(
            name="psA", bufs=1, space="PSUM"
        ) as psA:
            gpsA = [
                psA.tile([P, N], f32, name=f"gpsA{m}", tag=f"gpsA{m}") for m in range(HB)
            ]
            m01s = []
            for k in range(SB):
                geS = maskp.tile([P, N], f32, name="geS", tag="geS", bufs=2)
                geE = maskp.tile([P, N], f32, name="geE", tag="geE", bufs=2)
                m01 = maskp.tile([P, N], f32r, name="m01", tag="m01", bufs=4)
                nc.vector.tensor_scalar(
                    out=geS[:], in0=startb[:], scalar1=iota_f[:, k : k + 1],
                    scalar2=None, op0=ALU.is_le,
                )
                nc.vector.tensor_scalar(
                    out=geE[:], in0=end1b[:], scalar1=iota_f[:, k : k + 1],
                    scalar2=None, op0=ALU.is_le,
                )
                nc.vector.tensor_tensor(out=m01[:], in0=geS[:], in1=geE[:], op=ALU.subtract)
                m01s.append(m01)
                for m in range(HB):
                    nc.tensor.matmul(
                        gpsA[m][:, :], hid_sb[k][:, ts(m, P)], m01[:],
                        start=(k == 0), stop=(k == SB - 1),
                    )
            for m in range(HB):
                nc.vector.tensor_tensor(
                    out=rt[2 * HB + m][:, :], in0=gpsA[m][:, :], in1=recipb[:], op=ALU.mult
                )

        # =========== Phase B: hidden^T + proposal, rep^T gathered parts
        with tc.tile_pool(name="gathp", bufs=1) as gathp, tc.tile_pool(
            name="htp", bufs=2
        ) as htp, tc.tile_pool(name="propsb", bufs=2) as propsbp, tc.tile_pool(
            name="psB", bufs=1, space="PSUM"
        ) as psB:
            sh, eh, wh = [], [], []
            for j in range(NJ):
                g = gathp.tile([P, H], f32r, name=f"sh{j}", tag="sh", bufs=4)
                nc.gpsimd.indirect_dma_start(
                    out=g[:], out_offset=None, in_=hid_d[:, :],
                    in_offset=bass.IndirectOffsetOnAxis(ap=idx_sb[:, 0, j : j + 1], axis=0),
                )
                sh.append(g)
                g = gathp.tile([P, H], f32r, name=f"eh{j}", tag="eh", bufs=4)
                nc.gpsimd.indirect_dma_start(
                    out=g[:], out_offset=None, in_=hid_d[:, :],
                    in_offset=bass.IndirectOffsetOnAxis(ap=idx_sb[:, 1, j : j + 1], axis=0),
                )
                eh.append(g)
                g = gathp.tile([P, HW], f32r, name=f"wh{j}", tag="wh", bufs=4)
                nc.gpsimd.indirect_dma_start(
                    out=g[:], out_offset=None, in_=wemb_d[:, :],
                    in_offset=bass.IndirectOffsetOnAxis(ap=idx_sb[:, 2, j : j + 1], axis=0),
                )
                wh.append(g)

            # hidden^T n-groups + proposal
            for n in range(NJ):
                hts = [
                    htp.tile([P, N], f32r, name=f"ht{h}", tag=f"ht{h}")
                    for h in range(HB)
                ]
                for kk in range(NJ):
                    k = n * NJ + kk
                    for h in range(HB):
                        ptr = psB.tile([P, P], f32r, name="ptr", tag="tr", bufs=6)
                        nc.tensor.transpose(ptr[:], hid_sb[k][:, ts(h, P)], ident[:])
                        nc.vector.tensor_copy(hts[h][:, ts(kk, P)], ptr[:])
                prop_ps = psB.tile([N_BIO, N], f32, name="prop_ps", tag="prop", bufs=2)
                for h in range(HB):
                    nc.tensor.matmul(
                        prop_ps[:, :], wprop_sb[:, h, :], hts[h][:, :],
                        start=(h == 0), stop=(h == HB - 1),
                    )
                psb = propsbp.tile([N_BIO, N], f32, name="psb", tag="psb")
                nc.scalar.activation(
                    psb[:, :], prop_ps[:, :], AF.Identity, bias=bprop_sb[:, 0:1], scale=1.0
                )
                nc.scalar.dma_start(prop_o[:, ts(n, N)], psb[:, :])

            # rep^T gathered parts: start (k 0-5), end (6-11), width (18-20)
            for j in range(NJ):
                for k in range(HB):
                    ptr = psB.tile([P, P], f32r, name="ptr", tag="tr", bufs=6)
                    nc.tensor.transpose(ptr[:], sh[j][:, ts(k, P)], ident[:])
                    nc.vector.tensor_copy(rt[k][:, ts(j, P)], ptr[:])
                for k in range(HB):
                    ptr = psB.tile([P, P], f32r, name="ptr", tag="tr", bufs=6)
                    nc.tensor.transpose(ptr[:], eh[j][:, ts(k, P)], ident[:])
                    nc.vector.tensor_copy(rt[HB + k][:, ts(j, P)], ptr[:])
                for k in range(HW // P):
                    ptr = psB.tile([P, P], f32r, name="ptr", tag="tr", bufs=6)
                    nc.tensor.transpose(ptr[:], wh[j][:, ts(k, P)], ident[:])
                    nc.vector.tensor_copy(rt[3 * HB + k][:, ts(j, P)], ptr[:])

        # =========== Phase C: GEMMs + heads
        with tc.tile_pool(name="softp", bufs=1) as softp, tc.tile_pool(
            name="wsp", bufs=1
        ) as wsp, tc.tile_pool(name="psmm", bufs=1, space="PSUM") as psmm, tc.tile_pool(
            name="pshead", bufs=1, space="PSUM"
        ) as pshead:
            gps = [
                psmm.tile([P, N], f32, name=f"gps{m}", tag=f"gps{m}") for m in range(HB)
            ]

            # GEMM1: t^T = gelu(W_t1^T @ rep^T + b_t1)
            # weights stream on the SP ring in 2-k-chunk super-tiles to
            # amortize the slot-WAR round-trip latency
            for kg in range((KREP + 1) // 2):
                nk = min(2, KREP - kg * 2)
                wt = wsp.tile([P, 2, H], f32r, name="wt1s", tag="wt1s", bufs=5)
                nc.sync.dma_start(
                    wt[:, :nk, :],
                    bass.AP(
                        wt1_d.tensor, kg * 2 * P * H, [[H, P], [P * H, nk], [1, H]]
                    ),
                )
                for kk in range(nk):
                    k = kg * 2 + kk
                    for m in range(HB):
                        nc.tensor.matmul(
                            gps[m][:, :], wt[:, kk, ts(m, P)], rt[k][:, :],
                            start=(k == 0), stop=(k == KREP - 1),
                        )
            for m in range(HB):
                nc.scalar.activation(
                    tT[m][:, :], gps[m][:, :], AF.Gelu, bias=bt1_sb[:, m : m + 1], scale=1.0
                )

            # type head
            typeT_ps = pshead.tile([N_TYPE, N], f32, name="typeT_ps", tag="headT")
            for k in range(HB):
                nc.tensor.matmul(
                    typeT_ps[:, :], wt2_sb[:, k, :], tT[k][:, :],
                    start=(k == 0), stop=(k == HB - 1),
                )
            nc.scalar.activation(
                typeT_sb[:, :], typeT_ps[:, :], AF.Identity, bias=bt2_sb[:, 0:1], scale=1.0
            )
            nc.scalar.dma_start(type_o[:, :], typeT_sb[:, :])

            # softmax (span-major)
            for j in range(NJ):
                ttr = pshead.tile([P, N_TYPE], f32, name="ttr", tag="soft")
                nc.tensor.transpose(
                    ttr[:], typeT_sb[:, ts(j, P)], ident_f[:N_TYPE, :N_TYPE]
                )
                mx = softp.tile([P, 1], f32, name="mx", tag="mx", bufs=2)
                nc.vector.tensor_reduce(
                    out=mx[:], in_=ttr[:], axis=AX.X, op=ALU.max, negate=True
                )
                ex = softp.tile([P, N_TYPE], f32, name="ex", tag="ex", bufs=2)
                sm = softp.tile([P, 1], f32, name="sm", tag="sm", bufs=2)
                nc.scalar.activation(
                    ex[:], ttr[:], AF.Exp, bias=mx[:, 0:1], scale=1.0, accum_out=sm[:, 0:1]
                )
                rc = softp.tile([P, 1], f32, name="rc", tag="rc", bufs=2)
                nc.vector.reciprocal(rc[:], sm[:])
                nc.vector.tensor_scalar(
                    out=probs[j][:], in0=ex[:], scalar1=rc[:, 0:1], scalar2=None, op0=ALU.mult
                )

            # GEMM2 part A: rep^T rows (weights stream on SP ring after W_t1)
            for kg in range((KREP + 1) // 2):
                nk = min(2, KREP - kg * 2)
                ws = wsp.tile([P, 2, H], f32r, name="ws1s", tag="ws1s", bufs=5)
                nc.sync.dma_start(
                    ws[:, :nk, :],
                    bass.AP(
                        ws1_d.tensor, kg * 2 * P * H, [[H, P], [P * H, nk], [1, H]]
                    ),
                )
                for kk in range(nk):
                    k = kg * 2 + kk
                    for m in range(HB):
                        nc.tensor.matmul(
                            gps[m][:, :], ws[:, kk, ts(m, P)], rt[k][:, :],
                            start=(k == 0), stop=False,
                        )

            # probs^T chunk + final GEMM2 accumulation
            for j in range(NJ):
                ptr2 = pshead.tile([N_TYPE, P], f32, name="ptr2", tag="soft")
                nc.tensor.transpose(ptr2[:], probs[j][:, :], ident_f[:, :])
                nc.vector.tensor_copy(probsT[:, ts(j, P)], ptr2[:])
            ws_last = wsp.tile([P, H], f32r, name="wslast", tag="wslast", bufs=1)
            nc.sync.dma_start(ws_last[:N_TYPE, :], ws1_d[SPAN_DIM : SPAN_DIM + N_TYPE, :])
            for m in range(HB):
                nc.tensor.matmul(
                    gps[m][:, :], ws_last[:N_TYPE, ts(m, P)], probsT[:, :],
                    start=False, stop=True,
                )
            sT = [outs.tile([P, N], f32r, name=f"sT{m}", tag=f"hb{m}") for m in range(HB)]
            for m in range(HB):
                nc.scalar.activation(
                    sT[m][:, :], gps[m][:, :], AF.Gelu, bias=bs1_sb[:, m : m + 1], scale=1.0
                )

            # sensitivity head
            sensT_ps = pshead.tile([N_SENS, N], f32, name="sensT_ps", tag="headT")
            for k in range(HB):
                nc.tensor.matmul(
                    sensT_ps[:, :], ws2_sb[:, k, :], sT[k][:, :],
                    start=(k == 0), stop=(k == HB - 1),
                )
            nc.scalar.activation(
                sensT_sb[:, :], sensT_ps[:, :], AF.Identity, bias=bs2_sb[:, 0:1], scale=1.0
            )
            nc.scalar.dma_start(sens_o[:, :], sensT_sb[:, :])

    nc.compile()
    return nc


_NC_CACHE = None


def _get_program():
    global _NC_CACHE
    if _NC_CACHE is None:
        _NC_CACHE = _build_program()
    return _NC_CACHE


def _make_in_maps(inputs):
    hidden = np.asarray(inputs["hidden"], dtype=np.float32)
    spans = np.asarray(inputs["candidate_spans"], dtype=np.int64)
    width_emb = np.ascontiguousarray(np.asarray(inputs["width_emb"], np.float32))
    W_prop = np.ascontiguousarray(np.asarray(inputs["W_prop"], np.float32))
    b_prop = np.asarray(inputs["b_prop"], np.float32).reshape(N_BIO, 1)
    W_t1 = np.ascontiguousarray(np.asarray(inputs["W_t1"], np.float32))
    b_t1 = np.asarray(inputs["b_t1"], np.float32).reshape(HB, P)
    W_t2 = np.ascontiguousarray(np.asarray(inputs["W_t2"], np.float32))
    b_t2 = np.asarray(inputs["b_t2"], np.float32).reshape(N_TYPE, 1)
    W_s1 = np.ascontiguousarray(np.asarray(inputs["W_s1"], np.float32))
    b_s1 = np.asarray(inputs["b_s1"], np.float32).reshape(HB, P)
    W_s2 = np.ascontiguousarray(np.asarray(inputs["W_s2"], np.float32))
    b_s2 = np.asarray(inputs["b_s2"], np.float32).reshape(N_SENS, 1)

    starts = spans[..., 0]
    ends = spans[..., 1]
    widths = ends - starts + 1
    in_maps = []
    for b in range(B):
        gidx = np.stack(
            [starts[b], ends[b], np.minimum(widths[b], WIDTH_VOCAB - 1)]
        ).astype(np.int32)
        spanf = np.stack(
            [
                starts[b].astype(np.float32),
                (ends[b] + 1).astype(np.float32),
                (1.0 / widths[b]).astype(np.float32),
            ]
        ).astype(np.float32)
        in_maps.append(
            {
                "hidden": np.ascontiguousarray(hidden[b]),
                "gidx": gidx,
                "spanf": spanf,
                "width_emb": width_emb,
                "W_prop": W_prop,
                "b_prop": b_prop,
                "W_t1": W_t1,
                "b_t1": b_t1,
                "W_t2": W_t2,
                "b_t2": b_t2,
                "W_s1": W_s1,
                "b_s1": b_s1,
                "W_s2": W_s2,
                "b_s2": b_s2,
            }
        )
    return in_maps


def run(inputs, trace=False):
    nc = _get_program()
    in_maps = _make_in_maps(inputs)
    res = run_bass_kernel_spmd(nc, in_maps, core_ids=list(range(B)), trace=trace)
    prop = np.stack([res.results[b]["prop_out"].T for b in range(B)])
    typ = np.stack([res.results[b]["type_out"].T for b in range(B)])
    sens = np.stack([res.results[b]["sens_out"].T for b in range(B)])
    return (prop, typ, sens), res


def kernel(**inputs):
    out, _ = run(inputs, trace=False)
    return out
